# revision 12
# baseline (speedup 1.0000x reference)
"""Multi-head attention (16 heads, D=128) on 8 trn2 NeuronCores.

Sharding: tensor-parallel over heads — each core owns 2 heads.
Per core: qkv projection for its 768 channels (chan-major for q/k,
token-major for v), fused RMSNorm+RoPE on q/k, SDPA in transposed-score
layout (softmax partition reduction via ones-matmul on the PE), partial
proj over its 256 channels.  Host sums the 8 partial outputs + bias.

Matmul operands are fp16 (separate FWL weight load, full PE rate);
all accumulation is fp32 in PSUM; softmax statistics in fp32.
exp is computed as exp(s/sqrt(D) - 4) — the shift is softmax-invariant
and keeps fp16 exp values in range (no overflow).

Layouts (per core):
  xT       [C=2048, TOK=4096]  (x transposed on host; tokens = b*2048+n)
  w_qk     SBUF [128, 16, 512]  lhsT tiles; chan-tiles = [q_h0,q_h1,k_h0,k_h1]
  w_v      SBUF [128, 16, 256]  rhs tiles (token-major v production)
  qT/kT    SBUF [128, 2, 2048]  D-major per head, per batch
  v        SBUF [128, 16, 256]  token-major per batch
  exp      SBUF [128, 16, 512]  exp(scores^T) per 512-wide q-chunk
  attn_scr DRAM [256, 2048]     normalized attn out^T
  wpT      SBUF [128, 2, 2048]  proj rhs tiles
  y        DRAM [4096, 2048]    fp32 partial output (host adds cores + bias)
"""
import math
from contextlib import ExitStack

import numpy as np

import concourse.bass as bass
import concourse.mybir as mybir
import concourse.tile as tile
from concourse import bacc, bass_utils

F32 = mybir.dt.float32
F16 = mybir.dt.float16

H, D, B, N, C = 16, 128, 2, 2048, 2048
NCORES = 8
HPC = H // NCORES            # heads per core = 2
TOK = B * N                  # 4096
EPS = float(np.finfo(np.float32).eps)
SCALE = 1.0 / math.sqrt(D)
ESHIFT = -4.0                # exp(s*SCALE + ESHIFT); softmax-invariant

_CACHE = {}
RUN_KW = {}   # test.py sets {"trace": True}


def _pin_act_table():
    """Restrict Exp/Ln to the combined natural_log_exp_and_others set so the
    table-load pass keeps a single ACT table resident (the default greedy
    choice alternates exp_and_others <-> natural_log, ~2.7us per switch)."""
    import concourse.hw_specs as hw
    tabs = hw.get_activation_tables("gen3")
    for name, funcs in tabs.items():
        if name != "natural_log_exp_and_others":
            funcs.discard(mybir.ActivationFunctionType.Exp)
            funcs.discard(mybir.ActivationFunctionType.Ln)


def build_module():
    """Build + compile the per-core Bass module (same NEFF for all cores)."""
    if "nc" in _CACHE:
        return _CACHE["nc"]
    _pin_act_table()
    nc = bacc.Bacc("TRN2", target_bir_lowering=False, debug=False,
                   num_devices=NCORES)

    xt_h = nc.dram_tensor("xt", [C, TOK], F16, kind="ExternalInput")
    wqk_h = nc.dram_tensor("wqk", [C, 4 * 128], F16, kind="ExternalInput")
    wv_h = nc.dram_tensor("wv", [C, 2 * 128], F16, kind="ExternalInput")
    wp_h = nc.dram_tensor("wp", [2 * 128, C], F16, kind="ExternalInput")
    cos2_h = nc.dram_tensor("cos2", [128, N], F16, kind="ExternalInput")
    sin2_h = nc.dram_tensor("sin2", [128, N], F16, kind="ExternalInput")
    qkb_h = nc.dram_tensor("qkb", [128, 4], F32, kind="ExternalInput")
    vb_h = nc.dram_tensor("vb", [128, 256], F32, kind="ExternalInput")
    invg2_h = nc.dram_tensor("invg2", [128, 2], F16, kind="ExternalInput")
    onec_h = nc.dram_tensor("onec", [128, 1], F16, kind="ExternalInput")
    oner_h = nc.dram_tensor("oner", [1, 128], F16, kind="ExternalInput")
    twor_h = nc.dram_tensor("twor", [1, 128], F16, kind="ExternalInput")
    eps_h = nc.dram_tensor("eps", [1, 1], F32, kind="ExternalInput")
    nb4_h = nc.dram_tensor("nb4", [128, 1], F32, kind="ExternalInput")
    y_h = nc.dram_tensor("y", [TOK, C], F32, kind="ExternalOutput")

    with tile.TileContext(nc) as tc, ExitStack() as ctx:
        pc = ctx.enter_context(tc.tile_pool(name="consts", bufs=1))
        p_xt = ctx.enter_context(tc.tile_pool(name="xt", bufs=3))
        p_qkv = ctx.enter_context(tc.tile_pool(name="qkv", bufs=1))
        p_qraw = ctx.enter_context(tc.tile_pool(name="qraw", bufs=2))
        p_qsw = ctx.enter_context(tc.tile_pool(name="qsw", bufs=3))
        p_sq = ctx.enter_context(tc.tile_pool(name="sq", bufs=3))
        p_exp = ctx.enter_context(tc.tile_pool(name="exp", bufs=2))
        p_attn = ctx.enter_context(tc.tile_pool(name="attn", bufs=3))
        p_ao = ctx.enter_context(tc.tile_pool(name="ao", bufs=1))
        p_y = ctx.enter_context(tc.tile_pool(name="y", bufs=4))
        p_row = ctx.enter_context(tc.tile_pool(name="rows", bufs=4))
        p_ps = ctx.enter_context(tc.tile_pool(name="ps", bufs=8, space="PSUM"))

        # constants / weights (resident)
        wqk = pc.tile([128, 16, 512], F16)
        nc.sync.dma_start(wqk[:], wqk_h.ap().rearrange("(t p) j -> p t j", p=128))
        wv = pc.tile([128, 16, 256], F16)
        nc.sync.dma_start(wv[:], wv_h.ap().rearrange("(t p) j -> p t j", p=128))
        wp = pc.tile([128, 2, 2048], F16)
        nc.sync.dma_start(wp[:], wp_h.ap().rearrange("(t p) j -> p t j", p=128))
        cos2 = pc.tile([128, N], F16)
        nc.sync.dma_start(cos2[:], cos2_h.ap())
        sin2 = pc.tile([128, N], F16)
        nc.sync.dma_start(sin2[:], sin2_h.ap())
        qkb = pc.tile([128, 4], F32)
        nc.sync.dma_start(qkb[:], qkb_h.ap())
        vb = pc.tile([128, 256], F32)
        nc.sync.dma_start(vb[:], vb_h.ap())
        invg2 = pc.tile([128, 2], F16)
        nc.sync.dma_start(invg2[:], invg2_h.ap())
        onec = pc.tile([128, 1], F16)
        nc.sync.dma_start(onec[:], onec_h.ap())
        oner = pc.tile([1, 128], F16)
        nc.sync.dma_start(oner[:], oner_h.ap())
        twor = pc.tile([1, 128], F16)
        nc.sync.dma_start(twor[:], twor_h.ap())
        eps_t = pc.tile([1, 1], F32)
        nc.sync.dma_start(eps_t[:], eps_h.ap())
        nb4 = pc.tile([128, 1], F32)
        nc.sync.dma_start(nb4[:], nb4_h.ap())

        for b in range(B):
            # ======== stage A: qkv projection for batch b ========
            qT = p_qkv.tile([128, HPC, N], F16, tag="qT")
            kT = p_qkv.tile([128, HPC, N], F16, tag="kT")
            vtok = p_qkv.tile([128, 16, 256], F16, tag="v")
            qraw = None
            for ch in range(8):           # 256-token chunks
                tok0 = b * N + ch * 256
                if ch % 2 == 0:
                    qraw = p_qraw.tile([128, 4, 512], F16)
                off = (ch % 2) * 256
                ps_qk = [p_ps.tile([128, 256], F32, tag="ps", name=f"a{b}{ch}{ct}")
                         for ct in range(4)]
                ps_v = [p_ps.tile([128, 256], F32, tag="ps", name=f"av{b}{ch}{s}")
                        for s in range(2)]
                for half in range(2):
                    xt = p_xt.tile([128, 8, 256], F16)
                    src = xt_h.ap()[half * 1024:(half + 1) * 1024,
                                    tok0:tok0 + 256]
                    nc.sync.dma_start(xt[:], src.rearrange("(t p) j -> p t j", p=128))
                    for ct in range(4):
                        for kt in range(8):
                            nc.tensor.matmul(
                                ps_qk[ct][:], wqk[:, half * 8 + kt, ct * 128:(ct + 1) * 128],
                                xt[:, kt, :],
                                start=(half == 0 and kt == 0), stop=(half == 1 and kt == 7))
                    for s in range(2):
                        for kt in range(8):
                            nc.tensor.matmul(
                                ps_v[s][:], xt[:, kt, s * 128:(s + 1) * 128],
                                wv[:, half * 8 + kt, :],
                                start=(half == 0 and kt == 0), stop=(half == 1 and kt == 7))
                for ct in range(4):
                    nc.vector.tensor_scalar_add(qraw[:, ct, off:off + 256],
                                                ps_qk[ct][:], qkb[:, ct:ct + 1])
                for s in range(2):
                    nc.vector.tensor_add(out=vtok[:, ch * 2 + s, :],
                                         in0=ps_v[s][:], in1=vb[:])

                if ch % 2 == 1:
                    # ==== RMSNorm + RoPE on the completed 512-token group ====
                    g0 = (ch - 1) * 256   # within-batch token offset
                    for ct in range(4):
                        hl, is_k = ct % 2, ct // 2
                        dst = (kT if is_k else qT)
                        src_q = qraw[:, ct, :]
                        sq = p_sq.tile([128, 512], F16)
                        nc.vector.tensor_mul(out=sq[:], in0=src_q, in1=src_q)
                        ps_ss = p_ps.tile([1, 512], F32, tag="ps", name=f"ss{b}{ch}{ct}")
                        nc.tensor.matmul(ps_ss[:], invg2[:, is_k:is_k + 1], sq[:],
                                         start=True, stop=True)
                        # rrow = 1/sqrt(var+eps) = exp(-0.5*ln(var+eps))
                        lrow = p_row.tile([1, 512], F32, tag="lrow")
                        nc.scalar.activation(lrow[:], ps_ss[:],
                                             mybir.ActivationFunctionType.Ln,
                                             bias=eps_t[:], scale=1.0 / D)
                        rrow = p_row.tile([1, 512], F16, tag="recip")
                        nc.scalar.activation(rrow[:], lrow[:],
                                             mybir.ActivationFunctionType.Exp,
                                             scale=-0.5)
                        ps_rs = p_ps.tile([128, 512], F32, tag="ps", name=f"rs{b}{ch}{ct}")
                        nc.tensor.matmul(ps_rs[:], oner[:], rrow[:],
                                         start=True, stop=True)
                        qsw = p_qsw.tile([128, 512], F16)
                        nc.sync.dma_start(qsw[0:64, :], src_q[64:128, :])
                        nc.sync.dma_start(qsw[64:128, :], src_q[0:64, :])
                        # in-place: qc into qraw, qs into qsw
                        nc.vector.tensor_mul(out=src_q, in0=src_q,
                                             in1=cos2[:, g0:g0 + 512])
                        nc.vector.tensor_mul(out=qsw[:], in0=qsw[:],
                                             in1=sin2[:, g0:g0 + 512])
                        rot = dst[:, hl, g0:g0 + 512]
                        nc.vector.tensor_add(out=rot, in0=src_q, in1=qsw[:])
                        nc.vector.tensor_mul(out=rot, in0=rot, in1=ps_rs[:])

            # ======== SDPA for (b, h0) and (b, h1) ========
            ao = p_ao.tile([128, 2, N], F16)   # attn out^T, stays in SBUF
            pend = []                          # deferred normalize tails

            def normalize(hl, qc, ps_d, ps_av):
                q0 = qc * 512
                # rd = 1/d = exp(-ln(d))
                ld = p_row.tile([1, 512], F32, tag="ld", name=f"ld{b}{hl}{qc}")
                nc.scalar.activation(ld[:], ps_d[:],
                                     mybir.ActivationFunctionType.Ln)
                rd = p_row.tile([1, 512], F16, tag="rd", name=f"rd{b}{hl}{qc}")
                nc.scalar.activation(rd[:], ld[:],
                                     mybir.ActivationFunctionType.Exp,
                                     scale=-1.0)
                ps_bc = p_ps.tile([128, 512], F32, tag="ps", name=f"bc{b}{hl}{qc}")
                nc.tensor.matmul(ps_bc[:], oner[:], rd[:], start=True, stop=True)
                rb = p_attn.tile([128, 512], F32, tag="rb", name=f"rb{b}{hl}{qc}")
                nc.vector.tensor_copy(rb[:], ps_bc[:])
                nc.vector.tensor_mul(out=ao[:, hl, q0:q0 + 512],
                                     in0=ps_av[:], in1=rb[:])

            for hl in range(HPC):
                for qc in range(4):       # 512-wide q chunks
                    q0 = qc * 512
                    ex = p_exp.tile([128, 16, 512], F16)
                    ps_d = p_ps.tile([1, 512], F32, tag="ps", name=f"d{b}{hl}{qc}")
                    ps_av = p_ps.tile([128, 512], F32, tag="ps", name=f"o{b}{hl}{qc}")
                    # software-pipelined: QK runs LA tiles ahead so ACT exp
                    # latency hides behind PE work (PE queue is in-order)
                    LA = 2
                    ps_s = [None] * 16

                    def qk(kt):
                        ps_s[kt] = p_ps.tile([128, 512], F32, tag="ps",
                                             name=f"s{b}{hl}{qc}{kt}")
                        nc.tensor.matmul(ps_s[kt][:], kT[:, hl, kt * 128:(kt + 1) * 128],
                                         qT[:, hl, q0:q0 + 512], start=True, stop=True)

                    def tail(kt):
                        nc.scalar.activation(ex[:, kt, :], ps_s[kt][:],
                                             mybir.ActivationFunctionType.Exp,
                                             bias=nb4[:], scale=SCALE)
                        nc.tensor.matmul(ps_av[:], vtok[:, kt, hl * 128:(hl + 1) * 128],
                                         ex[:, kt, :],
                                         start=(kt == 0), stop=(kt == 15))

                    for kt in range(16):
                        qk(kt)
                        if kt == 6 and pend:
                            normalize(*pend.pop())
                        if kt >= LA:
                            tail(kt - LA)
                    for kt in range(16 - LA, 16):
                        tail(kt)
                    for kt in range(16):   # denominator sweep (exp tiles alive)
                        nc.tensor.matmul(ps_d[:], onec[:], ex[:, kt, :],
                                         start=(kt == 0), stop=(kt == 15))
                    pend.append((hl, qc, ps_d, ps_av))
            normalize(*pend.pop())

            # ======== stage C: partial proj for batch b ========
            for tt in range(16):          # 128-token tiles
                for oc in range(4):
                    ps_y = p_ps.tile([128, 512], F32, tag="ps", name=f"y{b}{tt}{oc}")
                    for ct in range(2):
                        nc.tensor.matmul(ps_y[:], ao[:, ct, tt * 128:(tt + 1) * 128],
                                         wp[:, ct, oc * 512:(oc + 1) * 512],
                                         start=(ct == 0), stop=(ct == 1))
                    yt = p_y.tile([128, 512], F32)
                    nc.vector.tensor_copy(yt[:], ps_y[:])
                    nc.sync.dma_start(
                        y_h.ap()[b * N + tt * 128:b * N + (tt + 1) * 128,
                                 oc * 512:(oc + 1) * 512], yt[:])

    nc.compile()
    _CACHE["nc"] = nc
    return nc


def make_in_maps(x, rope, qkv_w, qkv_b, proj_w, q_norm_w, k_norm_w):
    """Host-side prep: transpose x, slice/scale weights per core."""
    x = np.asarray(x, np.float32)
    rope = np.asarray(rope, np.float32)
    qkv_w = np.asarray(qkv_w, np.float32)
    qkv_b = np.asarray(qkv_b, np.float32)
    proj_w = np.asarray(proj_w, np.float32)
    g_q = np.asarray(q_norm_w, np.float32)
    g_k = np.asarray(k_norm_w, np.float32)
    if np.any(g_q == 0) or np.any(g_k == 0):
        raise ValueError("zero rmsnorm weight not supported")

    xt = np.ascontiguousarray(x.reshape(TOK, C).T.astype(np.float16))  # [C, TOK]
    cos = np.cos(rope)                                        # [N, 64]
    sin = np.sin(rope)
    cos2 = np.ascontiguousarray(
        np.concatenate([cos, cos], axis=1).T.astype(np.float16))       # [128, N]
    sin2 = np.ascontiguousarray(
        np.concatenate([-sin, sin], axis=1).T.astype(np.float16))      # [128, N]
    invg2 = np.stack([1.0 / g_q ** 2, 1.0 / g_k ** 2], axis=1).astype(np.float16)
    onec = np.ones((128, 1), np.float16)
    oner = np.ones((1, 128), np.float16)
    twor = np.full((1, 128), 2.0, np.float16)
    eps = np.full((1, 1), EPS, np.float32)
    nb4 = np.full((128, 1), ESHIFT, np.float32)

    in_maps = []
    for c in range(NCORES):
        hs = [HPC * c + hl for hl in range(HPC)]
        # chan-tiles: q_h0, q_h1, k_h0, k_h1 (g-scaled rows + bias)
        rows, biases = [], []
        for base, g in ((0, g_q), (C, g_k)):
            for h in hs:
                r0 = base + h * D
                rows.append(qkv_w[r0:r0 + D] * g[:, None])
                biases.append(qkv_b[r0:r0 + D] * g)
        wqk = np.ascontiguousarray(
            np.concatenate(rows, axis=0).T.astype(np.float16))           # [C, 512]
        qkb = np.stack(biases, axis=1)                                   # [128, 4]
        vrows = [qkv_w[2 * C + h * D:2 * C + (h + 1) * D] for h in hs]
        wv = np.ascontiguousarray(
            np.concatenate(vrows, axis=0).T.astype(np.float16))          # [C, 256]
        vbias = np.concatenate(
            [qkv_b[2 * C + h * D:2 * C + (h + 1) * D] for h in hs])      # [256]
        vb = np.broadcast_to(vbias, (128, 256)).astype(np.float32).copy()
        cols = np.concatenate([np.arange(h * D, (h + 1) * D) for h in hs])
        wpT = np.ascontiguousarray(proj_w[:, cols].T.astype(np.float16))  # [256, C]
        in_maps.append({
            "xt": xt, "wqk": wqk, "wv": wv, "wp": wpT,
            "cos2": cos2, "sin2": sin2, "qkb": qkb, "vb": vb,
            "invg2": invg2, "onec": onec, "oner": oner, "twor": twor,
            "eps": eps, "nb4": nb4,
        })
    return in_maps


def kernel(x, rope, qkv_w, qkv_b, proj_w, proj_b, q_norm_w, k_norm_w):
    nc = build_module()
    in_maps = make_in_maps(x, rope, qkv_w, qkv_b, proj_w, q_norm_w, k_norm_w)
    res = bass_utils.run_bass_kernel_spmd(nc, in_maps,
                                          core_ids=list(range(NCORES)), **RUN_KW)
    _CACHE["last_result"] = res
    y = np.zeros((TOK, C), np.float64)
    for c in range(NCORES):
        y += res.results[c]["y"].astype(np.float64)
    y += np.asarray(proj_b, np.float32).astype(np.float64)
    return y.astype(np.float32).reshape(B, N, C)


# revision 13
# speedup vs baseline: 1.1641x; 1.1641x over previous
"""Multi-head attention (16 heads, D=128) on 8 trn2 NeuronCores.

Sharding: tensor-parallel over heads — each core owns 2 heads.
Per core: qkv projection for its 768 channels (chan-major for q/k,
token-major for v), fused RMSNorm+RoPE on q/k, SDPA in transposed-score
layout (softmax partition reduction via ones-matmul on the PE), partial
proj over its 256 channels.  Host sums the 8 partial outputs + bias.

Matmul operands are fp16 (separate FWL weight load, full PE rate);
all accumulation is fp32 in PSUM; softmax statistics in fp32.
exp is computed as exp(s/sqrt(D) - 4) — the shift is softmax-invariant
and keeps fp16 exp values in range (no overflow).

Layouts (per core):
  xT       [C=2048, TOK=4096]  (x transposed on host; tokens = b*2048+n)
  w_qk     SBUF [128, 16, 512]  lhsT tiles; chan-tiles = [q_h0,q_h1,k_h0,k_h1]
  w_v      SBUF [128, 16, 256]  rhs tiles (token-major v production)
  qT/kT    SBUF [128, 2, 2048]  D-major per head, per batch
  v        SBUF [128, 16, 256]  token-major per batch
  exp      SBUF [128, 16, 512]  exp(scores^T) per 512-wide q-chunk
  attn_scr DRAM [256, 2048]     normalized attn out^T
  wpT      SBUF [128, 2, 2048]  proj rhs tiles
  y        DRAM [4096, 2048]    fp32 partial output (host adds cores + bias)
"""
import math
from contextlib import ExitStack

import numpy as np

import concourse.bass as bass
import concourse.mybir as mybir
import concourse.tile as tile
from concourse import bacc, bass_utils

F32 = mybir.dt.float32
F16 = mybir.dt.float16

H, D, B, N, C = 16, 128, 2, 2048, 2048
NCORES = 8
HPC = H // NCORES            # heads per core = 2
TOK = B * N                  # 4096
EPS = float(np.finfo(np.float32).eps)
SCALE = 1.0 / math.sqrt(D)
ESHIFT = -4.0                # exp(s*SCALE + ESHIFT); softmax-invariant

_CACHE = {}
RUN_KW = {}   # test.py sets {"trace": True}


def _pin_act_table():
    """Restrict Exp/Ln to the combined natural_log_exp_and_others set so the
    table-load pass keeps a single ACT table resident (the default greedy
    choice alternates exp_and_others <-> natural_log, ~2.7us per switch)."""
    import concourse.hw_specs as hw
    tabs = hw.get_activation_tables("gen3")
    for name, funcs in tabs.items():
        if name != "natural_log_exp_and_others":
            funcs.discard(mybir.ActivationFunctionType.Exp)
            funcs.discard(mybir.ActivationFunctionType.Ln)


def build_module():
    """Build + compile the per-core Bass module (same NEFF for all cores)."""
    if "nc" in _CACHE:
        return _CACHE["nc"]
    _pin_act_table()
    nc = bacc.Bacc("TRN2", target_bir_lowering=False, debug=False,
                   num_devices=NCORES)

    xt_h = nc.dram_tensor("xt", [C, TOK], F16, kind="ExternalInput")
    wqk_h = nc.dram_tensor("wqk", [C, 4 * 128], F16, kind="ExternalInput")
    wv_h = nc.dram_tensor("wv", [C, 2 * 128], F16, kind="ExternalInput")
    wp_h = nc.dram_tensor("wp", [2 * 128, C], F16, kind="ExternalInput")
    cos2_h = nc.dram_tensor("cos2", [128, N], F16, kind="ExternalInput")
    sin2_h = nc.dram_tensor("sin2", [128, N], F16, kind="ExternalInput")
    qkb_h = nc.dram_tensor("qkb", [128, 4], F32, kind="ExternalInput")
    vb_h = nc.dram_tensor("vb", [128, 256], F32, kind="ExternalInput")
    invg2_h = nc.dram_tensor("invg2", [128, 2], F16, kind="ExternalInput")
    onec_h = nc.dram_tensor("onec", [128, 1], F16, kind="ExternalInput")
    oner_h = nc.dram_tensor("oner", [1, 128], F16, kind="ExternalInput")
    twor_h = nc.dram_tensor("twor", [1, 128], F16, kind="ExternalInput")
    eps_h = nc.dram_tensor("eps", [1, 1], F32, kind="ExternalInput")
    nb4_h = nc.dram_tensor("nb4", [128, 1], F32, kind="ExternalInput")
    y_h = nc.dram_tensor("y", [TOK, C], F32, kind="ExternalOutput")

    with tile.TileContext(nc) as tc, ExitStack() as ctx:
        pc = ctx.enter_context(tc.tile_pool(name="consts", bufs=1))
        p_xt = ctx.enter_context(tc.tile_pool(name="xt", bufs=3))
        p_qkv = ctx.enter_context(tc.tile_pool(name="qkv", bufs=1))
        p_qraw = ctx.enter_context(tc.tile_pool(name="qraw", bufs=2))
        p_qsw = ctx.enter_context(tc.tile_pool(name="qsw", bufs=3))
        p_sq = ctx.enter_context(tc.tile_pool(name="sq", bufs=3))
        p_exp = ctx.enter_context(tc.tile_pool(name="exp", bufs=2))
        p_attn = ctx.enter_context(tc.tile_pool(name="attn", bufs=3))
        p_ao = ctx.enter_context(tc.tile_pool(name="ao", bufs=1))
        p_y = ctx.enter_context(tc.tile_pool(name="y", bufs=4))
        p_row = ctx.enter_context(tc.tile_pool(name="rows", bufs=4))
        p_ps = ctx.enter_context(tc.tile_pool(name="ps", bufs=8, space="PSUM"))

        # constants / weights (resident)
        wqk = pc.tile([128, 16, 512], F16)
        nc.sync.dma_start(wqk[:], wqk_h.ap().rearrange("(t p) j -> p t j", p=128))
        wv = pc.tile([128, 16, 256], F16)
        nc.sync.dma_start(wv[:], wv_h.ap().rearrange("(t p) j -> p t j", p=128))
        wp = pc.tile([128, 2, 2048], F16)
        nc.sync.dma_start(wp[:], wp_h.ap().rearrange("(t p) j -> p t j", p=128))
        cos2 = pc.tile([128, N], F16)
        nc.sync.dma_start(cos2[:], cos2_h.ap())
        sin2 = pc.tile([128, N], F16)
        nc.sync.dma_start(sin2[:], sin2_h.ap())
        qkb = pc.tile([128, 4], F32)
        nc.sync.dma_start(qkb[:], qkb_h.ap())
        vb = pc.tile([128, 256], F32)
        nc.sync.dma_start(vb[:], vb_h.ap())
        invg2 = pc.tile([128, 2], F16)
        nc.sync.dma_start(invg2[:], invg2_h.ap())
        onec = pc.tile([128, 1], F16)
        nc.sync.dma_start(onec[:], onec_h.ap())
        oner = pc.tile([1, 128], F16)
        nc.sync.dma_start(oner[:], oner_h.ap())
        twor = pc.tile([1, 128], F16)
        nc.sync.dma_start(twor[:], twor_h.ap())
        eps_t = pc.tile([1, 1], F32)
        nc.sync.dma_start(eps_t[:], eps_h.ap())
        nb4 = pc.tile([128, 1], F32)
        nc.sync.dma_start(nb4[:], nb4_h.ap())

        for b in range(B):
            # ======== stage A: qkv projection for batch b ========
            qT = p_qkv.tile([128, HPC, N], F16, tag="qT")
            kT = p_qkv.tile([128, HPC, N], F16, tag="kT")
            vtok = p_qkv.tile([128, 16, 256], F16, tag="v")
            qraw = None
            for ch in range(8):           # 256-token chunks
                tok0 = b * N + ch * 256
                if ch % 2 == 0:
                    qraw = p_qraw.tile([128, 4, 512], F16)
                off = (ch % 2) * 256
                ps_qk = [p_ps.tile([128, 256], F32, tag="ps", name=f"a{b}{ch}{ct}")
                         for ct in range(4)]
                ps_v = [p_ps.tile([128, 256], F32, tag="ps", name=f"av{b}{ch}{s}")
                        for s in range(2)]
                for half in range(2):
                    xt = p_xt.tile([128, 8, 256], F16)
                    src = xt_h.ap()[half * 1024:(half + 1) * 1024,
                                    tok0:tok0 + 256]
                    nc.sync.dma_start(xt[:], src.rearrange("(t p) j -> p t j", p=128))
                    for ct in range(4):
                        for kt in range(8):
                            nc.tensor.matmul(
                                ps_qk[ct][:], wqk[:, half * 8 + kt, ct * 128:(ct + 1) * 128],
                                xt[:, kt, :],
                                start=(half == 0 and kt == 0), stop=(half == 1 and kt == 7))
                    for s in range(2):
                        for kt in range(8):
                            nc.tensor.matmul(
                                ps_v[s][:], xt[:, kt, s * 128:(s + 1) * 128],
                                wv[:, half * 8 + kt, :],
                                start=(half == 0 and kt == 0), stop=(half == 1 and kt == 7))
                for ct in range(4):
                    nc.vector.tensor_scalar_add(qraw[:, ct, off:off + 256],
                                                ps_qk[ct][:], qkb[:, ct:ct + 1])
                for s in range(2):
                    nc.vector.tensor_add(out=vtok[:, ch * 2 + s, :],
                                         in0=ps_v[s][:], in1=vb[:])

                if ch % 2 == 1:
                    # ==== RMSNorm + RoPE on the completed 512-token group ====
                    g0 = (ch - 1) * 256   # within-batch token offset
                    for ct in range(4):
                        hl, is_k = ct % 2, ct // 2
                        dst = (kT if is_k else qT)
                        src_q = qraw[:, ct, :]
                        sq = p_sq.tile([128, 512], F16)
                        nc.vector.tensor_mul(out=sq[:], in0=src_q, in1=src_q)
                        ps_ss = p_ps.tile([1, 512], F32, tag="ps", name=f"ss{b}{ch}{ct}")
                        nc.tensor.matmul(ps_ss[:], invg2[:, is_k:is_k + 1], sq[:],
                                         start=True, stop=True)
                        # rrow = 1/sqrt(var+eps) = exp(-0.5*ln(var+eps))
                        lrow = p_row.tile([1, 512], F32, tag="lrow")
                        nc.scalar.activation(lrow[:], ps_ss[:],
                                             mybir.ActivationFunctionType.Ln,
                                             bias=eps_t[:], scale=1.0 / D)
                        rrow = p_row.tile([1, 512], F16, tag="recip")
                        nc.scalar.activation(rrow[:], lrow[:],
                                             mybir.ActivationFunctionType.Exp,
                                             scale=-0.5)
                        ps_rs = p_ps.tile([128, 512], F32, tag="ps", name=f"rs{b}{ch}{ct}")
                        nc.tensor.matmul(ps_rs[:], oner[:], rrow[:],
                                         start=True, stop=True)
                        qsw = p_qsw.tile([128, 512], F16)
                        nc.sync.dma_start(qsw[0:64, :], src_q[64:128, :])
                        nc.sync.dma_start(qsw[64:128, :], src_q[0:64, :])
                        # in-place: qc into qraw, qs into qsw
                        nc.vector.tensor_mul(out=src_q, in0=src_q,
                                             in1=cos2[:, g0:g0 + 512])
                        nc.vector.tensor_mul(out=qsw[:], in0=qsw[:],
                                             in1=sin2[:, g0:g0 + 512])
                        rot = dst[:, hl, g0:g0 + 512]
                        nc.vector.tensor_add(out=rot, in0=src_q, in1=qsw[:])
                        nc.vector.tensor_mul(out=rot, in0=rot, in1=ps_rs[:])

            # ======== SDPA for (b, h0) and (b, h1) ========
            ao = p_ao.tile([128, 2, N], F16)   # attn out^T, stays in SBUF
            pend = []                          # deferred normalize tails

            def normalize(hl, qc, ps_d, ps_av):
                q0 = qc * 512
                # rd = 1/d = exp(-ln(d))
                ld = p_row.tile([1, 512], F32, tag="ld", name=f"ld{b}{hl}{qc}")
                nc.scalar.activation(ld[:], ps_d[:],
                                     mybir.ActivationFunctionType.Ln)
                rd = p_row.tile([1, 512], F16, tag="rd", name=f"rd{b}{hl}{qc}")
                nc.scalar.activation(rd[:], ld[:],
                                     mybir.ActivationFunctionType.Exp,
                                     scale=-1.0)
                ps_bc = p_ps.tile([128, 512], F32, tag="ps", name=f"bc{b}{hl}{qc}")
                nc.tensor.matmul(ps_bc[:], oner[:], rd[:], start=True, stop=True)
                rb = p_attn.tile([128, 512], F32, tag="rb", name=f"rb{b}{hl}{qc}")
                nc.vector.tensor_copy(rb[:], ps_bc[:])
                nc.vector.tensor_mul(out=ao[:, hl, q0:q0 + 512],
                                     in0=ps_av[:], in1=rb[:])

            for hl in range(HPC):
                for qc in range(4):       # 512-wide q chunks
                    q0 = qc * 512
                    ex = p_exp.tile([128, 16, 512], F16)
                    ps_d = p_ps.tile([1, 512], F32, tag="ps", name=f"d{b}{hl}{qc}")
                    ps_av = p_ps.tile([128, 512], F32, tag="ps", name=f"o{b}{hl}{qc}")
                    # software-pipelined: QK runs LA tiles ahead so ACT exp
                    # latency hides behind PE work (PE queue is in-order)
                    LA = 2
                    ps_s = [None] * 16

                    def qk(kt):
                        ps_s[kt] = p_ps.tile([128, 512], F32, tag="ps",
                                             name=f"s{b}{hl}{qc}{kt}")
                        nc.tensor.matmul(ps_s[kt][:], kT[:, hl, kt * 128:(kt + 1) * 128],
                                         qT[:, hl, q0:q0 + 512], start=True, stop=True)

                    def tail(kt):
                        nc.scalar.activation(ex[:, kt, :], ps_s[kt][:],
                                             mybir.ActivationFunctionType.Exp,
                                             bias=nb4[:], scale=SCALE)
                        nc.tensor.matmul(ps_av[:], vtok[:, kt, hl * 128:(hl + 1) * 128],
                                         ex[:, kt, :],
                                         start=(kt == 0), stop=(kt == 15))
                        nc.tensor.matmul(ps_d[:], onec[:], ex[:, kt, :],
                                         start=(kt == 0), stop=(kt == 15))

                    for kt in range(16):
                        qk(kt)
                        if kt == 6 and pend:
                            normalize(*pend.pop())
                        if kt >= LA:
                            tail(kt - LA)
                    for kt in range(16 - LA, 16):
                        tail(kt)
                    pend.append((hl, qc, ps_d, ps_av))
            normalize(*pend.pop())

            # ======== stage C: partial proj for batch b ========
            for tt in range(16):          # 128-token tiles
                for oc in range(4):
                    ps_y = p_ps.tile([128, 512], F32, tag="ps", name=f"y{b}{tt}{oc}")
                    for ct in range(2):
                        nc.tensor.matmul(ps_y[:], ao[:, ct, tt * 128:(tt + 1) * 128],
                                         wp[:, ct, oc * 512:(oc + 1) * 512],
                                         start=(ct == 0), stop=(ct == 1))
                    yt = p_y.tile([128, 512], F32)
                    nc.vector.tensor_copy(yt[:], ps_y[:])
                    nc.sync.dma_start(
                        y_h.ap()[b * N + tt * 128:b * N + (tt + 1) * 128,
                                 oc * 512:(oc + 1) * 512], yt[:])

    nc.compile()
    _CACHE["nc"] = nc
    return nc


def make_in_maps(x, rope, qkv_w, qkv_b, proj_w, q_norm_w, k_norm_w):
    """Host-side prep: transpose x, slice/scale weights per core."""
    x = np.asarray(x, np.float32)
    rope = np.asarray(rope, np.float32)
    qkv_w = np.asarray(qkv_w, np.float32)
    qkv_b = np.asarray(qkv_b, np.float32)
    proj_w = np.asarray(proj_w, np.float32)
    g_q = np.asarray(q_norm_w, np.float32)
    g_k = np.asarray(k_norm_w, np.float32)
    if np.any(g_q == 0) or np.any(g_k == 0):
        raise ValueError("zero rmsnorm weight not supported")

    xt = np.ascontiguousarray(x.reshape(TOK, C).T.astype(np.float16))  # [C, TOK]
    cos = np.cos(rope)                                        # [N, 64]
    sin = np.sin(rope)
    cos2 = np.ascontiguousarray(
        np.concatenate([cos, cos], axis=1).T.astype(np.float16))       # [128, N]
    sin2 = np.ascontiguousarray(
        np.concatenate([-sin, sin], axis=1).T.astype(np.float16))      # [128, N]
    invg2 = np.stack([1.0 / g_q ** 2, 1.0 / g_k ** 2], axis=1).astype(np.float16)
    onec = np.ones((128, 1), np.float16)
    oner = np.ones((1, 128), np.float16)
    twor = np.full((1, 128), 2.0, np.float16)
    eps = np.full((1, 1), EPS, np.float32)
    nb4 = np.full((128, 1), ESHIFT, np.float32)

    in_maps = []
    for c in range(NCORES):
        hs = [HPC * c + hl for hl in range(HPC)]
        # chan-tiles: q_h0, q_h1, k_h0, k_h1 (g-scaled rows + bias)
        rows, biases = [], []
        for base, g in ((0, g_q), (C, g_k)):
            for h in hs:
                r0 = base + h * D
                rows.append(qkv_w[r0:r0 + D] * g[:, None])
                biases.append(qkv_b[r0:r0 + D] * g)
        wqk = np.ascontiguousarray(
            np.concatenate(rows, axis=0).T.astype(np.float16))           # [C, 512]
        qkb = np.stack(biases, axis=1)                                   # [128, 4]
        vrows = [qkv_w[2 * C + h * D:2 * C + (h + 1) * D] for h in hs]
        wv = np.ascontiguousarray(
            np.concatenate(vrows, axis=0).T.astype(np.float16))          # [C, 256]
        vbias = np.concatenate(
            [qkv_b[2 * C + h * D:2 * C + (h + 1) * D] for h in hs])      # [256]
        vb = np.broadcast_to(vbias, (128, 256)).astype(np.float32).copy()
        cols = np.concatenate([np.arange(h * D, (h + 1) * D) for h in hs])
        wpT = np.ascontiguousarray(proj_w[:, cols].T.astype(np.float16))  # [256, C]
        in_maps.append({
            "xt": xt, "wqk": wqk, "wv": wv, "wp": wpT,
            "cos2": cos2, "sin2": sin2, "qkb": qkb, "vb": vb,
            "invg2": invg2, "onec": onec, "oner": oner, "twor": twor,
            "eps": eps, "nb4": nb4,
        })
    return in_maps


def kernel(x, rope, qkv_w, qkv_b, proj_w, proj_b, q_norm_w, k_norm_w):
    nc = build_module()
    in_maps = make_in_maps(x, rope, qkv_w, qkv_b, proj_w, q_norm_w, k_norm_w)
    res = bass_utils.run_bass_kernel_spmd(nc, in_maps,
                                          core_ids=list(range(NCORES)), **RUN_KW)
    _CACHE["last_result"] = res
    y = np.zeros((TOK, C), np.float64)
    for c in range(NCORES):
        y += res.results[c]["y"].astype(np.float64)
    y += np.asarray(proj_b, np.float32).astype(np.float64)
    return y.astype(np.float32).reshape(B, N, C)


# revision 15
# speedup vs baseline: 1.2653x; 1.0869x over previous
"""Multi-head attention (16 heads, D=128) on 8 trn2 NeuronCores.

Sharding: tensor-parallel over heads — each core owns 2 heads.
Per core: qkv projection for its 768 channels (chan-major for q/k,
token-major for v), fused RMSNorm+RoPE on q/k, SDPA in transposed-score
layout (softmax partition reduction via ones-matmul on the PE), partial
proj over its 256 channels.  Host sums the 8 partial outputs + bias.

Matmul operands are fp16 (separate FWL weight load, full PE rate);
all accumulation is fp32 in PSUM; softmax statistics in fp32.
exp is computed as exp(s/sqrt(D) - 4) — the shift is softmax-invariant
and keeps fp16 exp values in range (no overflow).

Layouts (per core):
  xT       [C=2048, TOK=4096]  (x transposed on host; tokens = b*2048+n)
  w_qk     SBUF [128, 16, 512]  lhsT tiles; chan-tiles = [q_h0,q_h1,k_h0,k_h1]
  w_v      SBUF [128, 16, 256]  rhs tiles (token-major v production)
  qT/kT    SBUF [128, 2, 2048]  D-major per head, per batch
  v        SBUF [128, 16, 256]  token-major per batch
  exp      SBUF [128, 16, 512]  exp(scores^T) per 512-wide q-chunk
  attn_scr DRAM [256, 2048]     normalized attn out^T
  wpT      SBUF [128, 2, 2048]  proj rhs tiles
  y        DRAM [4096, 2048]    fp32 partial output (host adds cores + bias)
"""
import math
from contextlib import ExitStack

import numpy as np

import concourse.bass as bass
import concourse.mybir as mybir
import concourse.tile as tile
from concourse import bacc, bass_utils

F32 = mybir.dt.float32
F16 = mybir.dt.float16

H, D, B, N, C = 16, 128, 2, 2048, 2048
NCORES = 8
HPC = H // NCORES            # heads per core = 2
TOK = B * N                  # 4096
EPS = float(np.finfo(np.float32).eps)
SCALE = 1.0 / math.sqrt(D)
ESHIFT = -4.0                # exp(s*SCALE + ESHIFT); softmax-invariant

_CACHE = {}
RUN_KW = {}   # test.py sets {"trace": True}


def _pin_act_table():
    """Restrict Exp/Ln to the combined natural_log_exp_and_others set so the
    table-load pass keeps a single ACT table resident (the default greedy
    choice alternates exp_and_others <-> natural_log, ~2.7us per switch)."""
    import concourse.hw_specs as hw
    tabs = hw.get_activation_tables("gen3")
    for name, funcs in tabs.items():
        if name != "natural_log_exp_and_others":
            funcs.discard(mybir.ActivationFunctionType.Exp)
            funcs.discard(mybir.ActivationFunctionType.Ln)


def build_module():
    """Build + compile the per-core Bass module (same NEFF for all cores)."""
    if "nc" in _CACHE:
        return _CACHE["nc"]
    _pin_act_table()
    nc = bacc.Bacc("TRN2", target_bir_lowering=False, debug=False,
                   num_devices=NCORES)

    xt_h = nc.dram_tensor("xt", [C, TOK], F16, kind="ExternalInput")
    wqk_h = nc.dram_tensor("wqk", [C, 4 * 128], F16, kind="ExternalInput")
    wv_h = nc.dram_tensor("wv", [C, 2 * 128], F16, kind="ExternalInput")
    wp_h = nc.dram_tensor("wp", [2 * 128, C], F16, kind="ExternalInput")
    cos2_h = nc.dram_tensor("cos2", [128, N], F16, kind="ExternalInput")
    sin2_h = nc.dram_tensor("sin2", [128, N], F16, kind="ExternalInput")
    qkb_h = nc.dram_tensor("qkb", [128, 4], F32, kind="ExternalInput")
    vb_h = nc.dram_tensor("vb", [128, 256], F32, kind="ExternalInput")
    invg2_h = nc.dram_tensor("invg2", [128, 2], F16, kind="ExternalInput")
    onec_h = nc.dram_tensor("onec", [128, 1], F16, kind="ExternalInput")
    oner_h = nc.dram_tensor("oner", [1, 128], F16, kind="ExternalInput")
    twor_h = nc.dram_tensor("twor", [1, 128], F16, kind="ExternalInput")
    eps_h = nc.dram_tensor("eps", [1, 1], F32, kind="ExternalInput")
    nb4_h = nc.dram_tensor("nb4", [128, 1], F32, kind="ExternalInput")
    y_h = nc.dram_tensor("y", [TOK, C], F32, kind="ExternalOutput")

    with tile.TileContext(nc) as tc, ExitStack() as ctx:
        pc = ctx.enter_context(tc.tile_pool(name="consts", bufs=1))
        p_xt = ctx.enter_context(tc.tile_pool(name="xt", bufs=3))
        p_qkv = ctx.enter_context(tc.tile_pool(name="qkv", bufs=1))
        p_qraw = ctx.enter_context(tc.tile_pool(name="qraw", bufs=2))
        p_qsw = ctx.enter_context(tc.tile_pool(name="qsw", bufs=3))
        p_sq = ctx.enter_context(tc.tile_pool(name="sq", bufs=3))
        p_exp = ctx.enter_context(tc.tile_pool(name="exp", bufs=2))
        p_attn = ctx.enter_context(tc.tile_pool(name="attn", bufs=3))
        p_ao = ctx.enter_context(tc.tile_pool(name="ao", bufs=1))
        p_y = ctx.enter_context(tc.tile_pool(name="y", bufs=4))
        p_row = ctx.enter_context(tc.tile_pool(name="rows", bufs=4))
        p_ps = ctx.enter_context(tc.tile_pool(name="ps", bufs=8, space="PSUM"))

        # constants / weights (resident)
        wqk = pc.tile([128, 16, 512], F16)
        nc.sync.dma_start(wqk[:], wqk_h.ap().rearrange("(t p) j -> p t j", p=128))
        wv = pc.tile([128, 16, 256], F16)
        nc.sync.dma_start(wv[:], wv_h.ap().rearrange("(t p) j -> p t j", p=128))
        wp = pc.tile([128, 2, 2048], F16)
        nc.sync.dma_start(wp[:], wp_h.ap().rearrange("(t p) j -> p t j", p=128))
        cos2 = pc.tile([128, N], F16)
        nc.sync.dma_start(cos2[:], cos2_h.ap())
        sin2 = pc.tile([128, N], F16)
        nc.sync.dma_start(sin2[:], sin2_h.ap())
        qkb = pc.tile([128, 4], F32)
        nc.sync.dma_start(qkb[:], qkb_h.ap())
        vb = pc.tile([128, 256], F32)
        nc.sync.dma_start(vb[:], vb_h.ap())
        invg2 = pc.tile([128, 2], F16)
        nc.sync.dma_start(invg2[:], invg2_h.ap())
        onec = pc.tile([128, 1], F16)
        nc.sync.dma_start(onec[:], onec_h.ap())
        oner = pc.tile([1, 128], F16)
        nc.sync.dma_start(oner[:], oner_h.ap())
        twor = pc.tile([1, 128], F16)
        nc.sync.dma_start(twor[:], twor_h.ap())
        eps_t = pc.tile([1, 1], F32)
        nc.sync.dma_start(eps_t[:], eps_h.ap())
        nb4 = pc.tile([128, 1], F32)
        nc.sync.dma_start(nb4[:], nb4_h.ap())

        for b in range(B):
            # ======== stage A: qkv projection for batch b ========
            qT = p_qkv.tile([128, HPC, N], F16, tag="qT")
            kT = p_qkv.tile([128, HPC, N], F16, tag="kT")
            vtok = p_qkv.tile([128, 16, 256], F16, tag="v")
            qraw = None
            ph1_pend = []   # deferred sq/sumsq/ln/exp of the previous group
            ph2_pend = []   # deferred rs-broadcast + rope of the previous group

            def ph1(qraw_g, g0, gi):
                rrows = []
                for ct in range(4):
                    is_k = ct // 2
                    src_q = qraw_g[:, ct, :]
                    sq = p_sq.tile([128, 512], F16, tag="sq", name=f"sq{b}{gi}{ct}")
                    nc.vector.tensor_mul(out=sq[:], in0=src_q, in1=src_q)
                    ps_ss = p_ps.tile([1, 512], F32, tag="ps", name=f"ss{b}{gi}{ct}")
                    nc.tensor.matmul(ps_ss[:], invg2[:, is_k:is_k + 1], sq[:],
                                     start=True, stop=True)
                    # rrow = 1/sqrt(var+eps) = exp(-0.5*ln(var+eps))
                    lrow = p_row.tile([1, 512], F32, tag="lrow", name=f"lr{b}{gi}{ct}")
                    nc.scalar.activation(lrow[:], ps_ss[:],
                                         mybir.ActivationFunctionType.Ln,
                                         bias=eps_t[:], scale=1.0 / D)
                    rrow = p_row.tile([1, 512], F16, tag="recip", name=f"rr{b}{gi}{ct}")
                    nc.scalar.activation(rrow[:], lrow[:],
                                         mybir.ActivationFunctionType.Exp,
                                         scale=-0.5)
                    rrows.append(rrow)
                return rrows

            def ph2(qraw_g, g0, gi, rrows):
                for ct in range(4):
                    hl, is_k = ct % 2, ct // 2
                    dst = (kT if is_k else qT)
                    src_q = qraw_g[:, ct, :]
                    ps_rs = p_ps.tile([128, 512], F32, tag="ps", name=f"rs{b}{gi}{ct}")
                    nc.tensor.matmul(ps_rs[:], oner[:], rrows[ct][:],
                                     start=True, stop=True)
                    qsw = p_qsw.tile([128, 512], F16, tag="qsw", name=f"qsw{b}{gi}{ct}")
                    nc.sync.dma_start(qsw[0:64, :], src_q[64:128, :])
                    nc.sync.dma_start(qsw[64:128, :], src_q[0:64, :])
                    # in-place: qc into qraw, qs into qsw
                    nc.vector.tensor_mul(out=src_q, in0=src_q,
                                         in1=cos2[:, g0:g0 + 512])
                    nc.vector.tensor_mul(out=qsw[:], in0=qsw[:],
                                         in1=sin2[:, g0:g0 + 512])
                    rot = dst[:, hl, g0:g0 + 512]
                    nc.vector.tensor_add(out=rot, in0=src_q, in1=qsw[:])
                    nc.vector.tensor_mul(out=rot, in0=rot, in1=ps_rs[:])

            for ch in range(8):           # 256-token chunks
                tok0 = b * N + ch * 256
                if ch % 2 == 0:
                    qraw = p_qraw.tile([128, 4, 512], F16)
                off = (ch % 2) * 256
                ps_qk = [p_ps.tile([128, 256], F32, tag="ps", name=f"a{b}{ch}{ct}")
                         for ct in range(4)]
                ps_v = [p_ps.tile([128, 256], F32, tag="ps", name=f"av{b}{ch}{s}")
                        for s in range(2)]
                for half in range(2):
                    xt = p_xt.tile([128, 8, 256], F16)
                    src = xt_h.ap()[half * 1024:(half + 1) * 1024,
                                    tok0:tok0 + 256]
                    nc.sync.dma_start(xt[:], src.rearrange("(t p) j -> p t j", p=128))
                    for ct in range(4):
                        for kt in range(8):
                            nc.tensor.matmul(
                                ps_qk[ct][:], wqk[:, half * 8 + kt, ct * 128:(ct + 1) * 128],
                                xt[:, kt, :],
                                start=(half == 0 and kt == 0), stop=(half == 1 and kt == 7))
                    for s in range(2):
                        for kt in range(8):
                            nc.tensor.matmul(
                                ps_v[s][:], xt[:, kt, s * 128:(s + 1) * 128],
                                wv[:, half * 8 + kt, :],
                                start=(half == 0 and kt == 0), stop=(half == 1 and kt == 7))
                    # inject deferred epilogues mid-stream so their PE/ACT
                    # latency hides behind this chunk's dense matmuls
                    if half == 0 and ph1_pend:
                        args = ph1_pend.pop()
                        ph2_pend.append((args[0], args[1], args[2], ph1(*args)))
                    elif half == 1 and ph2_pend:
                        ph2(*ph2_pend.pop())
                for ct in range(4):
                    nc.vector.tensor_scalar_add(qraw[:, ct, off:off + 256],
                                                ps_qk[ct][:], qkb[:, ct:ct + 1])
                for s in range(2):
                    nc.vector.tensor_add(out=vtok[:, ch * 2 + s, :],
                                         in0=ps_v[s][:], in1=vb[:])
                if ch % 2 == 1:
                    ph1_pend.append((qraw, (ch - 1) * 256, ch // 2))
            # flush the last group's epilogue
            args = ph1_pend.pop()
            ph2(args[0], args[1], args[2], ph1(*args))

            # ======== SDPA for (b, h0) and (b, h1) ========
            ao = p_ao.tile([128, 2, N], F16)   # attn out^T, stays in SBUF
            pend = []                          # deferred normalize tails

            def normalize(hl, qc, ps_d, ps_av):
                q0 = qc * 512
                # rd = 1/d = exp(-ln(d))
                ld = p_row.tile([1, 512], F32, tag="ld", name=f"ld{b}{hl}{qc}")
                nc.scalar.activation(ld[:], ps_d[:],
                                     mybir.ActivationFunctionType.Ln)
                rd = p_row.tile([1, 512], F16, tag="rd", name=f"rd{b}{hl}{qc}")
                nc.scalar.activation(rd[:], ld[:],
                                     mybir.ActivationFunctionType.Exp,
                                     scale=-1.0)
                ps_bc = p_ps.tile([128, 512], F32, tag="ps", name=f"bc{b}{hl}{qc}")
                nc.tensor.matmul(ps_bc[:], oner[:], rd[:], start=True, stop=True)
                rb = p_attn.tile([128, 512], F32, tag="rb", name=f"rb{b}{hl}{qc}")
                nc.vector.tensor_copy(rb[:], ps_bc[:])
                nc.vector.tensor_mul(out=ao[:, hl, q0:q0 + 512],
                                     in0=ps_av[:], in1=rb[:])

            for hl in range(HPC):
                for qc in range(4):       # 512-wide q chunks
                    q0 = qc * 512
                    ex = p_exp.tile([128, 16, 512], F16)
                    ps_d = p_ps.tile([1, 512], F32, tag="ps", name=f"d{b}{hl}{qc}")
                    ps_av = p_ps.tile([128, 512], F32, tag="ps", name=f"o{b}{hl}{qc}")
                    # software-pipelined: QK runs LA tiles ahead so ACT exp
                    # latency hides behind PE work (PE queue is in-order)
                    LA = 2
                    ps_s = [None] * 16

                    def qk(kt):
                        ps_s[kt] = p_ps.tile([128, 512], F32, tag="ps",
                                             name=f"s{b}{hl}{qc}{kt}")
                        nc.tensor.matmul(ps_s[kt][:], kT[:, hl, kt * 128:(kt + 1) * 128],
                                         qT[:, hl, q0:q0 + 512], start=True, stop=True)

                    def tail(kt):
                        nc.scalar.activation(ex[:, kt, :], ps_s[kt][:],
                                             mybir.ActivationFunctionType.Exp,
                                             bias=nb4[:], scale=SCALE)
                        nc.tensor.matmul(ps_av[:], vtok[:, kt, hl * 128:(hl + 1) * 128],
                                         ex[:, kt, :],
                                         start=(kt == 0), stop=(kt == 15))
                        nc.tensor.matmul(ps_d[:], onec[:], ex[:, kt, :],
                                         start=(kt == 0), stop=(kt == 15))

                    for kt in range(16):
                        qk(kt)
                        if kt == 6 and pend:
                            normalize(*pend.pop())
                        if kt >= LA:
                            tail(kt - LA)
                    for kt in range(16 - LA, 16):
                        tail(kt)
                    pend.append((hl, qc, ps_d, ps_av))
            normalize(*pend.pop())

            # ======== stage C: partial proj for batch b ========
            for tt in range(16):          # 128-token tiles
                for oc in range(4):
                    ps_y = p_ps.tile([128, 512], F32, tag="ps", name=f"y{b}{tt}{oc}")
                    for ct in range(2):
                        nc.tensor.matmul(ps_y[:], ao[:, ct, tt * 128:(tt + 1) * 128],
                                         wp[:, ct, oc * 512:(oc + 1) * 512],
                                         start=(ct == 0), stop=(ct == 1))
                    yt = p_y.tile([128, 512], F32)
                    nc.vector.tensor_copy(yt[:], ps_y[:])
                    nc.sync.dma_start(
                        y_h.ap()[b * N + tt * 128:b * N + (tt + 1) * 128,
                                 oc * 512:(oc + 1) * 512], yt[:])

    nc.compile()
    _CACHE["nc"] = nc
    return nc


def make_in_maps(x, rope, qkv_w, qkv_b, proj_w, q_norm_w, k_norm_w):
    """Host-side prep: transpose x, slice/scale weights per core."""
    x = np.asarray(x, np.float32)
    rope = np.asarray(rope, np.float32)
    qkv_w = np.asarray(qkv_w, np.float32)
    qkv_b = np.asarray(qkv_b, np.float32)
    proj_w = np.asarray(proj_w, np.float32)
    g_q = np.asarray(q_norm_w, np.float32)
    g_k = np.asarray(k_norm_w, np.float32)
    if np.any(g_q == 0) or np.any(g_k == 0):
        raise ValueError("zero rmsnorm weight not supported")

    xt = np.ascontiguousarray(x.reshape(TOK, C).T.astype(np.float16))  # [C, TOK]
    cos = np.cos(rope)                                        # [N, 64]
    sin = np.sin(rope)
    cos2 = np.ascontiguousarray(
        np.concatenate([cos, cos], axis=1).T.astype(np.float16))       # [128, N]
    sin2 = np.ascontiguousarray(
        np.concatenate([-sin, sin], axis=1).T.astype(np.float16))      # [128, N]
    invg2 = np.stack([1.0 / g_q ** 2, 1.0 / g_k ** 2], axis=1).astype(np.float16)
    onec = np.ones((128, 1), np.float16)
    oner = np.ones((1, 128), np.float16)
    twor = np.full((1, 128), 2.0, np.float16)
    eps = np.full((1, 1), EPS, np.float32)
    nb4 = np.full((128, 1), ESHIFT, np.float32)

    in_maps = []
    for c in range(NCORES):
        hs = [HPC * c + hl for hl in range(HPC)]
        # chan-tiles: q_h0, q_h1, k_h0, k_h1 (g-scaled rows + bias)
        rows, biases = [], []
        for base, g in ((0, g_q), (C, g_k)):
            for h in hs:
                r0 = base + h * D
                rows.append(qkv_w[r0:r0 + D] * g[:, None])
                biases.append(qkv_b[r0:r0 + D] * g)
        wqk = np.ascontiguousarray(
            np.concatenate(rows, axis=0).T.astype(np.float16))           # [C, 512]
        qkb = np.stack(biases, axis=1)                                   # [128, 4]
        vrows = [qkv_w[2 * C + h * D:2 * C + (h + 1) * D] for h in hs]
        wv = np.ascontiguousarray(
            np.concatenate(vrows, axis=0).T.astype(np.float16))          # [C, 256]
        vbias = np.concatenate(
            [qkv_b[2 * C + h * D:2 * C + (h + 1) * D] for h in hs])      # [256]
        vb = np.broadcast_to(vbias, (128, 256)).astype(np.float32).copy()
        cols = np.concatenate([np.arange(h * D, (h + 1) * D) for h in hs])
        wpT = np.ascontiguousarray(proj_w[:, cols].T.astype(np.float16))  # [256, C]
        in_maps.append({
            "xt": xt, "wqk": wqk, "wv": wv, "wp": wpT,
            "cos2": cos2, "sin2": sin2, "qkb": qkb, "vb": vb,
            "invg2": invg2, "onec": onec, "oner": oner, "twor": twor,
            "eps": eps, "nb4": nb4,
        })
    return in_maps


def kernel(x, rope, qkv_w, qkv_b, proj_w, proj_b, q_norm_w, k_norm_w):
    nc = build_module()
    in_maps = make_in_maps(x, rope, qkv_w, qkv_b, proj_w, q_norm_w, k_norm_w)
    res = bass_utils.run_bass_kernel_spmd(nc, in_maps,
                                          core_ids=list(range(NCORES)), **RUN_KW)
    _CACHE["last_result"] = res
    y = np.zeros((TOK, C), np.float64)
    for c in range(NCORES):
        y += res.results[c]["y"].astype(np.float64)
    y += np.asarray(proj_b, np.float32).astype(np.float64)
    return y.astype(np.float32).reshape(B, N, C)


# revision 16
# speedup vs baseline: 1.2673x; 1.0016x over previous
"""Multi-head attention (16 heads, D=128) on 8 trn2 NeuronCores.

Sharding: tensor-parallel over heads — each core owns 2 heads.
Per core: qkv projection for its 768 channels (chan-major for q/k,
token-major for v), fused RMSNorm+RoPE on q/k, SDPA in transposed-score
layout (softmax partition reduction via ones-matmul on the PE), partial
proj over its 256 channels.  Host sums the 8 partial outputs + bias.

Matmul operands are fp16 (separate FWL weight load, full PE rate);
all accumulation is fp32 in PSUM; softmax statistics in fp32.
exp is computed as exp(s/sqrt(D) - 4) — the shift is softmax-invariant
and keeps fp16 exp values in range (no overflow).

Layouts (per core):
  xT       [C=2048, TOK=4096]  (x transposed on host; tokens = b*2048+n)
  w_qk     SBUF [128, 16, 512]  lhsT tiles; chan-tiles = [q_h0,q_h1,k_h0,k_h1]
  w_v      SBUF [128, 16, 256]  rhs tiles (token-major v production)
  qT/kT    SBUF [128, 2, 2048]  D-major per head, per batch
  v        SBUF [128, 16, 256]  token-major per batch
  exp      SBUF [128, 16, 512]  exp(scores^T) per 512-wide q-chunk
  attn_scr DRAM [256, 2048]     normalized attn out^T
  wpT      SBUF [128, 2, 2048]  proj rhs tiles
  y        DRAM [4096, 2048]    fp32 partial output (host adds cores + bias)
"""
import math
from contextlib import ExitStack

import numpy as np

import concourse.bass as bass
import concourse.mybir as mybir
import concourse.tile as tile
from concourse import bacc, bass_utils

F32 = mybir.dt.float32
F16 = mybir.dt.float16

H, D, B, N, C = 16, 128, 2, 2048, 2048
NCORES = 8
HPC = H // NCORES            # heads per core = 2
TOK = B * N                  # 4096
EPS = float(np.finfo(np.float32).eps)
SCALE = 1.0 / math.sqrt(D)
ESHIFT = -4.0                # exp(s*SCALE + ESHIFT); softmax-invariant

_CACHE = {}
RUN_KW = {}   # test.py sets {"trace": True}


def _pin_act_table():
    """Restrict Exp/Ln to the combined natural_log_exp_and_others set so the
    table-load pass keeps a single ACT table resident (the default greedy
    choice alternates exp_and_others <-> natural_log, ~2.7us per switch)."""
    import concourse.hw_specs as hw
    tabs = hw.get_activation_tables("gen3")
    for name, funcs in tabs.items():
        if name != "natural_log_exp_and_others":
            funcs.discard(mybir.ActivationFunctionType.Exp)
            funcs.discard(mybir.ActivationFunctionType.Ln)


def build_module():
    """Build + compile the per-core Bass module (same NEFF for all cores)."""
    if "nc" in _CACHE:
        return _CACHE["nc"]
    _pin_act_table()
    nc = bacc.Bacc("TRN2", target_bir_lowering=False, debug=False,
                   num_devices=NCORES)

    xt_h = nc.dram_tensor("xt", [C, TOK], F16, kind="ExternalInput")
    wqk_h = nc.dram_tensor("wqk", [C, 4 * 128], F16, kind="ExternalInput")
    wv_h = nc.dram_tensor("wv", [C, 2 * 128], F16, kind="ExternalInput")
    wp_h = nc.dram_tensor("wp", [2 * 128, C], F16, kind="ExternalInput")
    cos2_h = nc.dram_tensor("cos2", [128, N], F16, kind="ExternalInput")
    sin2_h = nc.dram_tensor("sin2", [128, N], F16, kind="ExternalInput")
    qkb_h = nc.dram_tensor("qkb", [128, 4], F32, kind="ExternalInput")
    vb_h = nc.dram_tensor("vb", [128, 256], F32, kind="ExternalInput")
    invg2_h = nc.dram_tensor("invg2", [128, 2], F16, kind="ExternalInput")
    onec_h = nc.dram_tensor("onec", [128, 1], F16, kind="ExternalInput")
    oner_h = nc.dram_tensor("oner", [1, 128], F16, kind="ExternalInput")
    twor_h = nc.dram_tensor("twor", [1, 128], F16, kind="ExternalInput")
    eps_h = nc.dram_tensor("eps", [1, 1], F32, kind="ExternalInput")
    nb4_h = nc.dram_tensor("nb4", [128, 1], F32, kind="ExternalInput")
    y_h = nc.dram_tensor("y", [TOK, C], F32, kind="ExternalOutput")

    with tile.TileContext(nc) as tc, ExitStack() as ctx:
        pc = ctx.enter_context(tc.tile_pool(name="consts", bufs=1))
        p_xt = ctx.enter_context(tc.tile_pool(name="xt", bufs=3))
        p_qkv = ctx.enter_context(tc.tile_pool(name="qkv", bufs=1))
        p_qraw = ctx.enter_context(tc.tile_pool(name="qraw", bufs=2))
        p_qsw = ctx.enter_context(tc.tile_pool(name="qsw", bufs=3))
        p_sq = ctx.enter_context(tc.tile_pool(name="sq", bufs=3))
        p_exp = ctx.enter_context(tc.tile_pool(name="exp", bufs=2))
        p_attn = ctx.enter_context(tc.tile_pool(name="attn", bufs=3))
        p_ao = ctx.enter_context(tc.tile_pool(name="ao", bufs=1))
        p_y = ctx.enter_context(tc.tile_pool(name="y", bufs=4))
        p_row = ctx.enter_context(tc.tile_pool(name="rows", bufs=4))
        p_ps = ctx.enter_context(tc.tile_pool(name="ps", bufs=8, space="PSUM"))

        # constants / weights (resident)
        wqk = pc.tile([128, 16, 512], F16)
        wv = pc.tile([128, 16, 256], F16)
        for hf in range(2):
            sl = slice(hf * 1024, (hf + 1) * 1024)
            nc.sync.dma_start(wqk[:, hf * 8:(hf + 1) * 8, :],
                              wqk_h.ap()[sl].rearrange("(t p) j -> p t j", p=128))
            nc.sync.dma_start(wv[:, hf * 8:(hf + 1) * 8, :],
                              wv_h.ap()[sl].rearrange("(t p) j -> p t j", p=128))
        wp = pc.tile([128, 2, 2048], F16)
        nc.sync.dma_start(wp[:], wp_h.ap().rearrange("(t p) j -> p t j", p=128))
        cos2 = pc.tile([128, N], F16)
        nc.sync.dma_start(cos2[:], cos2_h.ap())
        sin2 = pc.tile([128, N], F16)
        nc.sync.dma_start(sin2[:], sin2_h.ap())
        qkb = pc.tile([128, 4], F32)
        nc.sync.dma_start(qkb[:], qkb_h.ap())
        vb = pc.tile([128, 256], F32)
        nc.sync.dma_start(vb[:], vb_h.ap())
        invg2 = pc.tile([128, 2], F16)
        nc.sync.dma_start(invg2[:], invg2_h.ap())
        onec = pc.tile([128, 1], F16)
        nc.sync.dma_start(onec[:], onec_h.ap())
        oner = pc.tile([1, 128], F16)
        nc.sync.dma_start(oner[:], oner_h.ap())
        twor = pc.tile([1, 128], F16)
        nc.sync.dma_start(twor[:], twor_h.ap())
        eps_t = pc.tile([1, 1], F32)
        nc.sync.dma_start(eps_t[:], eps_h.ap())
        nb4 = pc.tile([128, 1], F32)
        nc.sync.dma_start(nb4[:], nb4_h.ap())

        for b in range(B):
            # ======== stage A: qkv projection for batch b ========
            qT = p_qkv.tile([128, HPC, N], F16, tag="qT")
            kT = p_qkv.tile([128, HPC, N], F16, tag="kT")
            vtok = p_qkv.tile([128, 16, 256], F16, tag="v")
            qraw = None
            ph1_pend = []   # deferred sq/sumsq/ln/exp of the previous group
            ph2_pend = []   # deferred rs-broadcast + rope of the previous group

            def ph1(qraw_g, g0, gi):
                rrows = []
                for ct in range(4):
                    is_k = ct // 2
                    src_q = qraw_g[:, ct, :]
                    sq = p_sq.tile([128, 512], F16, tag="sq", name=f"sq{b}{gi}{ct}")
                    nc.vector.tensor_mul(out=sq[:], in0=src_q, in1=src_q)
                    ps_ss = p_ps.tile([1, 512], F32, tag="ps", name=f"ss{b}{gi}{ct}")
                    nc.tensor.matmul(ps_ss[:], invg2[:, is_k:is_k + 1], sq[:],
                                     start=True, stop=True)
                    # rrow = 1/sqrt(var+eps) = exp(-0.5*ln(var+eps))
                    lrow = p_row.tile([1, 512], F32, tag="lrow", name=f"lr{b}{gi}{ct}")
                    nc.scalar.activation(lrow[:], ps_ss[:],
                                         mybir.ActivationFunctionType.Ln,
                                         bias=eps_t[:], scale=1.0 / D)
                    rrow = p_row.tile([1, 512], F16, tag="recip", name=f"rr{b}{gi}{ct}")
                    nc.scalar.activation(rrow[:], lrow[:],
                                         mybir.ActivationFunctionType.Exp,
                                         scale=-0.5)
                    rrows.append(rrow)
                return rrows

            def ph2(qraw_g, g0, gi, rrows):
                for ct in range(4):
                    hl, is_k = ct % 2, ct // 2
                    dst = (kT if is_k else qT)
                    src_q = qraw_g[:, ct, :]
                    ps_rs = p_ps.tile([128, 512], F32, tag="ps", name=f"rs{b}{gi}{ct}")
                    nc.tensor.matmul(ps_rs[:], oner[:], rrows[ct][:],
                                     start=True, stop=True)
                    qsw = p_qsw.tile([128, 512], F16, tag="qsw", name=f"qsw{b}{gi}{ct}")
                    nc.sync.dma_start(qsw[0:64, :], src_q[64:128, :])
                    nc.sync.dma_start(qsw[64:128, :], src_q[0:64, :])
                    # in-place: qc into qraw, qs into qsw
                    nc.vector.tensor_mul(out=src_q, in0=src_q,
                                         in1=cos2[:, g0:g0 + 512])
                    nc.vector.tensor_mul(out=qsw[:], in0=qsw[:],
                                         in1=sin2[:, g0:g0 + 512])
                    rot = dst[:, hl, g0:g0 + 512]
                    nc.vector.tensor_add(out=rot, in0=src_q, in1=qsw[:])
                    nc.vector.tensor_mul(out=rot, in0=rot, in1=ps_rs[:])

            for ch in range(8):           # 256-token chunks
                tok0 = b * N + ch * 256
                if ch % 2 == 0:
                    qraw = p_qraw.tile([128, 4, 512], F16)
                off = (ch % 2) * 256
                ps_qk = [p_ps.tile([128, 256], F32, tag="ps", name=f"a{b}{ch}{ct}")
                         for ct in range(4)]
                ps_v = [p_ps.tile([128, 256], F32, tag="ps", name=f"av{b}{ch}{s}")
                        for s in range(2)]
                for half in range(2):
                    xt = p_xt.tile([128, 8, 256], F16)
                    src = xt_h.ap()[half * 1024:(half + 1) * 1024,
                                    tok0:tok0 + 256]
                    nc.sync.dma_start(xt[:], src.rearrange("(t p) j -> p t j", p=128))
                    for ct in range(4):
                        for kt in range(8):
                            nc.tensor.matmul(
                                ps_qk[ct][:], wqk[:, half * 8 + kt, ct * 128:(ct + 1) * 128],
                                xt[:, kt, :],
                                start=(half == 0 and kt == 0), stop=(half == 1 and kt == 7))
                    for s in range(2):
                        for kt in range(8):
                            nc.tensor.matmul(
                                ps_v[s][:], xt[:, kt, s * 128:(s + 1) * 128],
                                wv[:, half * 8 + kt, :],
                                start=(half == 0 and kt == 0), stop=(half == 1 and kt == 7))
                    # inject deferred epilogues mid-stream so their PE/ACT
                    # latency hides behind this chunk's dense matmuls
                    if half == 0 and ph1_pend:
                        args = ph1_pend.pop()
                        ph2_pend.append((args[0], args[1], args[2], ph1(*args)))
                    elif half == 1 and ph2_pend:
                        ph2(*ph2_pend.pop())
                for ct in range(4):
                    nc.vector.tensor_scalar_add(qraw[:, ct, off:off + 256],
                                                ps_qk[ct][:], qkb[:, ct:ct + 1])
                for s in range(2):
                    nc.vector.tensor_add(out=vtok[:, ch * 2 + s, :],
                                         in0=ps_v[s][:], in1=vb[:])
                if ch % 2 == 1:
                    ph1_pend.append((qraw, (ch - 1) * 256, ch // 2))
            # flush the last group's epilogue
            args = ph1_pend.pop()
            ph2(args[0], args[1], args[2], ph1(*args))

            # ======== SDPA for (b, h0) and (b, h1) ========
            ao = p_ao.tile([128, 2, N], F16)   # attn out^T, stays in SBUF
            pend = []                          # deferred normalize tails

            def normalize(hl, qc, ps_d, ps_av):
                q0 = qc * 512
                # rd = 1/d = exp(-ln(d))
                ld = p_row.tile([1, 512], F32, tag="ld", name=f"ld{b}{hl}{qc}")
                nc.scalar.activation(ld[:], ps_d[:],
                                     mybir.ActivationFunctionType.Ln)
                rd = p_row.tile([1, 512], F16, tag="rd", name=f"rd{b}{hl}{qc}")
                nc.scalar.activation(rd[:], ld[:],
                                     mybir.ActivationFunctionType.Exp,
                                     scale=-1.0)
                ps_bc = p_ps.tile([128, 512], F32, tag="ps", name=f"bc{b}{hl}{qc}")
                nc.tensor.matmul(ps_bc[:], oner[:], rd[:], start=True, stop=True)
                rb = p_attn.tile([128, 512], F32, tag="rb", name=f"rb{b}{hl}{qc}")
                nc.vector.tensor_copy(rb[:], ps_bc[:])
                nc.vector.tensor_mul(out=ao[:, hl, q0:q0 + 512],
                                     in0=ps_av[:], in1=rb[:])

            for hl in range(HPC):
                for qc in range(4):       # 512-wide q chunks
                    q0 = qc * 512
                    ex = p_exp.tile([128, 16, 512], F16)
                    ps_d = p_ps.tile([1, 512], F32, tag="ps", name=f"d{b}{hl}{qc}")
                    ps_av = p_ps.tile([128, 512], F32, tag="ps", name=f"o{b}{hl}{qc}")
                    # software-pipelined: QK pairs run one pair ahead so ACT
                    # exp latency hides behind PE work (PE queue is in-order)
                    ps_s = [None] * 16

                    def qk(kt):
                        ps_s[kt] = p_ps.tile([128, 512], F32, tag="ps",
                                             name=f"s{b}{hl}{qc}{kt}")
                        nc.tensor.matmul(ps_s[kt][:], kT[:, hl, kt * 128:(kt + 1) * 128],
                                         qT[:, hl, q0:q0 + 512], start=True, stop=True)

                    def tailpair(j):
                        nc.scalar.activation(ex[:, j, :], ps_s[j][:],
                                             mybir.ActivationFunctionType.Exp,
                                             bias=nb4[:], scale=SCALE)
                        nc.scalar.activation(ex[:, j + 1, :], ps_s[j + 1][:],
                                             mybir.ActivationFunctionType.Exp,
                                             bias=nb4[:], scale=SCALE)
                        for kt in (j, j + 1):
                            nc.tensor.matmul(ps_av[:],
                                             vtok[:, kt, hl * 128:(hl + 1) * 128],
                                             ex[:, kt, :],
                                             start=(kt == 0), stop=(kt == 15))
                        for kt in (j, j + 1):
                            nc.tensor.matmul(ps_d[:], onec[:], ex[:, kt, :],
                                             start=(kt == 0), stop=(kt == 15))

                    for i in range(8):
                        qk(2 * i)
                        qk(2 * i + 1)
                        if i == 3 and pend:
                            normalize(*pend.pop())
                        if i >= 1:
                            tailpair(2 * i - 2)
                    tailpair(14)
                    pend.append((hl, qc, ps_d, ps_av))
            normalize(*pend.pop())

            # ======== stage C: partial proj for batch b ========
            for tt in range(16):          # 128-token tiles
                for oc in range(4):
                    ps_y = p_ps.tile([128, 512], F32, tag="ps", name=f"y{b}{tt}{oc}")
                    for ct in range(2):
                        nc.tensor.matmul(ps_y[:], ao[:, ct, tt * 128:(tt + 1) * 128],
                                         wp[:, ct, oc * 512:(oc + 1) * 512],
                                         start=(ct == 0), stop=(ct == 1))
                    yt = p_y.tile([128, 512], F32)
                    if oc % 2 == 0:
                        nc.vector.tensor_copy(yt[:], ps_y[:])
                    else:
                        nc.scalar.copy(yt[:], ps_y[:])
                    nc.sync.dma_start(
                        y_h.ap()[b * N + tt * 128:b * N + (tt + 1) * 128,
                                 oc * 512:(oc + 1) * 512], yt[:])

    nc.compile()
    _CACHE["nc"] = nc
    return nc


def make_in_maps(x, rope, qkv_w, qkv_b, proj_w, q_norm_w, k_norm_w):
    """Host-side prep: transpose x, slice/scale weights per core."""
    x = np.asarray(x, np.float32)
    rope = np.asarray(rope, np.float32)
    qkv_w = np.asarray(qkv_w, np.float32)
    qkv_b = np.asarray(qkv_b, np.float32)
    proj_w = np.asarray(proj_w, np.float32)
    g_q = np.asarray(q_norm_w, np.float32)
    g_k = np.asarray(k_norm_w, np.float32)
    if np.any(g_q == 0) or np.any(g_k == 0):
        raise ValueError("zero rmsnorm weight not supported")

    xt = np.ascontiguousarray(x.reshape(TOK, C).T.astype(np.float16))  # [C, TOK]
    cos = np.cos(rope)                                        # [N, 64]
    sin = np.sin(rope)
    cos2 = np.ascontiguousarray(
        np.concatenate([cos, cos], axis=1).T.astype(np.float16))       # [128, N]
    sin2 = np.ascontiguousarray(
        np.concatenate([-sin, sin], axis=1).T.astype(np.float16))      # [128, N]
    invg2 = np.stack([1.0 / g_q ** 2, 1.0 / g_k ** 2], axis=1).astype(np.float16)
    onec = np.ones((128, 1), np.float16)
    oner = np.ones((1, 128), np.float16)
    twor = np.full((1, 128), 2.0, np.float16)
    eps = np.full((1, 1), EPS, np.float32)
    nb4 = np.full((128, 1), ESHIFT, np.float32)

    in_maps = []
    for c in range(NCORES):
        hs = [HPC * c + hl for hl in range(HPC)]
        # chan-tiles: q_h0, q_h1, k_h0, k_h1 (g-scaled rows + bias)
        rows, biases = [], []
        for base, g in ((0, g_q), (C, g_k)):
            for h in hs:
                r0 = base + h * D
                rows.append(qkv_w[r0:r0 + D] * g[:, None])
                biases.append(qkv_b[r0:r0 + D] * g)
        wqk = np.ascontiguousarray(
            np.concatenate(rows, axis=0).T.astype(np.float16))           # [C, 512]
        qkb = np.stack(biases, axis=1)                                   # [128, 4]
        vrows = [qkv_w[2 * C + h * D:2 * C + (h + 1) * D] for h in hs]
        wv = np.ascontiguousarray(
            np.concatenate(vrows, axis=0).T.astype(np.float16))          # [C, 256]
        vbias = np.concatenate(
            [qkv_b[2 * C + h * D:2 * C + (h + 1) * D] for h in hs])      # [256]
        vb = np.broadcast_to(vbias, (128, 256)).astype(np.float32).copy()
        cols = np.concatenate([np.arange(h * D, (h + 1) * D) for h in hs])
        wpT = np.ascontiguousarray(proj_w[:, cols].T.astype(np.float16))  # [256, C]
        in_maps.append({
            "xt": xt, "wqk": wqk, "wv": wv, "wp": wpT,
            "cos2": cos2, "sin2": sin2, "qkb": qkb, "vb": vb,
            "invg2": invg2, "onec": onec, "oner": oner, "twor": twor,
            "eps": eps, "nb4": nb4,
        })
    return in_maps


def kernel(x, rope, qkv_w, qkv_b, proj_w, proj_b, q_norm_w, k_norm_w):
    nc = build_module()
    in_maps = make_in_maps(x, rope, qkv_w, qkv_b, proj_w, q_norm_w, k_norm_w)
    res = bass_utils.run_bass_kernel_spmd(nc, in_maps,
                                          core_ids=list(range(NCORES)), **RUN_KW)
    _CACHE["last_result"] = res
    y = np.zeros((TOK, C), np.float64)
    for c in range(NCORES):
        y += res.results[c]["y"].astype(np.float64)
    y += np.asarray(proj_b, np.float32).astype(np.float64)
    return y.astype(np.float32).reshape(B, N, C)


# revision 17
# speedup vs baseline: 1.4000x; 1.1047x over previous
"""Multi-head attention (16 heads, D=128) on 8 trn2 NeuronCores.

Sharding: tensor-parallel over heads — each core owns 2 heads.
Per core: qkv projection for its 768 channels (chan-major for q/k,
token-major for v), fused RMSNorm+RoPE on q/k, SDPA in transposed-score
layout (softmax partition reduction via ones-matmul on the PE), partial
proj over its 256 channels.  Host sums the 8 partial outputs + bias.

Matmul operands are fp16 (separate FWL weight load, full PE rate);
all accumulation is fp32 in PSUM; softmax statistics in fp32.
exp is computed as exp(s/sqrt(D) - 4) — the shift is softmax-invariant
and keeps fp16 exp values in range (no overflow).

Layouts (per core):
  xT       [C=2048, TOK=4096]  (x transposed on host; tokens = b*2048+n)
  w_qk     SBUF [128, 16, 512]  lhsT tiles; chan-tiles = [q_h0,q_h1,k_h0,k_h1]
  w_v      SBUF [128, 16, 256]  rhs tiles (token-major v production)
  qT/kT    SBUF [128, 2, 2048]  D-major per head, per batch
  v        SBUF [128, 16, 256]  token-major per batch
  exp      SBUF [128, 16, 512]  exp(scores^T) per 512-wide q-chunk
  attn_scr DRAM [256, 2048]     normalized attn out^T
  wpT      SBUF [128, 2, 2048]  proj rhs tiles
  y        DRAM [4096, 2048]    fp32 partial output (host adds cores + bias)
"""
import math
from contextlib import ExitStack

import numpy as np

import concourse.bass as bass
import concourse.mybir as mybir
import concourse.tile as tile
from concourse import bacc, bass_utils

F32 = mybir.dt.float32
F16 = mybir.dt.float16

H, D, B, N, C = 16, 128, 2, 2048, 2048
NCORES = 8
HPC = H // NCORES            # heads per core = 2
TOK = B * N                  # 4096
EPS = float(np.finfo(np.float32).eps)
SCALE = 1.0 / math.sqrt(D)
ESHIFT = -4.0                # exp(s*SCALE + ESHIFT); softmax-invariant

_CACHE = {}
RUN_KW = {}   # test.py sets {"trace": True}


def _pin_act_table():
    """Restrict Exp/Ln to the combined natural_log_exp_and_others set so the
    table-load pass keeps a single ACT table resident (the default greedy
    choice alternates exp_and_others <-> natural_log, ~2.7us per switch)."""
    import concourse.hw_specs as hw
    tabs = hw.get_activation_tables("gen3")
    for name, funcs in tabs.items():
        if name != "natural_log_exp_and_others":
            funcs.discard(mybir.ActivationFunctionType.Exp)
            funcs.discard(mybir.ActivationFunctionType.Ln)


def build_module():
    """Build + compile the per-core Bass module (same NEFF for all cores)."""
    if "nc" in _CACHE:
        return _CACHE["nc"]
    _pin_act_table()
    nc = bacc.Bacc("TRN2", target_bir_lowering=False, debug=False,
                   num_devices=NCORES)

    xt_h = nc.dram_tensor("xt", [C, TOK], F16, kind="ExternalInput")
    wqk_h = nc.dram_tensor("wqk", [C, 4 * 128], F16, kind="ExternalInput")
    wv_h = nc.dram_tensor("wv", [C, 2 * 128], F16, kind="ExternalInput")
    wp_h = nc.dram_tensor("wp", [2 * 128, C], F16, kind="ExternalInput")
    cos2_h = nc.dram_tensor("cos2", [128, N], F16, kind="ExternalInput")
    sin2_h = nc.dram_tensor("sin2", [128, N], F16, kind="ExternalInput")
    qkb_h = nc.dram_tensor("qkb", [128, 4], F32, kind="ExternalInput")
    vb_h = nc.dram_tensor("vb", [128, 256], F32, kind="ExternalInput")
    invg2_h = nc.dram_tensor("invg2", [128, 2], F16, kind="ExternalInput")
    onec_h = nc.dram_tensor("onec", [128, 128], F16, kind="ExternalInput")
    oner_h = nc.dram_tensor("oner", [1, 128], F16, kind="ExternalInput")
    twor_h = nc.dram_tensor("twor", [1, 128], F16, kind="ExternalInput")
    eps_h = nc.dram_tensor("eps", [1, 1], F32, kind="ExternalInput")
    nb4_h = nc.dram_tensor("nb4", [128, 1], F32, kind="ExternalInput")
    y_h = nc.dram_tensor("y", [TOK, C], F32, kind="ExternalOutput")

    with tile.TileContext(nc) as tc, ExitStack() as ctx:
        pc = ctx.enter_context(tc.tile_pool(name="consts", bufs=1))
        p_xt = ctx.enter_context(tc.tile_pool(name="xt", bufs=3))
        p_qkv = ctx.enter_context(tc.tile_pool(name="qkv", bufs=1))
        p_qraw = ctx.enter_context(tc.tile_pool(name="qraw", bufs=2))
        p_qsw = ctx.enter_context(tc.tile_pool(name="qsw", bufs=3))
        p_sq = ctx.enter_context(tc.tile_pool(name="sq", bufs=3))
        p_exp = ctx.enter_context(tc.tile_pool(name="exp", bufs=2))
        p_attn = ctx.enter_context(tc.tile_pool(name="attn", bufs=3))
        p_ao = ctx.enter_context(tc.tile_pool(name="ao", bufs=1))
        p_y = ctx.enter_context(tc.tile_pool(name="y", bufs=4))
        p_row = ctx.enter_context(tc.tile_pool(name="rows", bufs=4))
        p_ps = ctx.enter_context(tc.tile_pool(name="ps", bufs=8, space="PSUM"))

        # constants / weights (resident)
        wqk = pc.tile([128, 16, 512], F16)
        wv = pc.tile([128, 16, 256], F16)
        for hf in range(2):
            sl = slice(hf * 1024, (hf + 1) * 1024)
            nc.sync.dma_start(wqk[:, hf * 8:(hf + 1) * 8, :],
                              wqk_h.ap()[sl].rearrange("(t p) j -> p t j", p=128))
            nc.sync.dma_start(wv[:, hf * 8:(hf + 1) * 8, :],
                              wv_h.ap()[sl].rearrange("(t p) j -> p t j", p=128))
        wp = pc.tile([128, 2, 2048], F16)
        nc.sync.dma_start(wp[:], wp_h.ap().rearrange("(t p) j -> p t j", p=128))
        cos2 = pc.tile([128, N], F16)
        nc.sync.dma_start(cos2[:], cos2_h.ap())
        sin2 = pc.tile([128, N], F16)
        nc.sync.dma_start(sin2[:], sin2_h.ap())
        qkb = pc.tile([128, 4], F32)
        nc.sync.dma_start(qkb[:], qkb_h.ap())
        vb = pc.tile([128, 256], F32)
        nc.sync.dma_start(vb[:], vb_h.ap())
        invg2 = pc.tile([128, 2], F16)
        nc.sync.dma_start(invg2[:], invg2_h.ap())
        onec = pc.tile([128, 128], F16)
        nc.sync.dma_start(onec[:], onec_h.ap())
        oner = pc.tile([1, 128], F16)
        nc.sync.dma_start(oner[:], oner_h.ap())
        twor = pc.tile([1, 128], F16)
        nc.sync.dma_start(twor[:], twor_h.ap())
        eps_t = pc.tile([1, 1], F32)
        nc.sync.dma_start(eps_t[:], eps_h.ap())
        nb4 = pc.tile([128, 1], F32)
        nc.sync.dma_start(nb4[:], nb4_h.ap())

        for b in range(B):
            # ======== stage A: qkv projection for batch b ========
            qT = p_qkv.tile([128, HPC, N], F16, tag="qT")
            kT = p_qkv.tile([128, HPC, N], F16, tag="kT")
            vtok = p_qkv.tile([128, 16, 256], F16, tag="v")
            qraw = None
            ph1_pend = []   # deferred sq/sumsq/ln/exp of the previous group
            ph2_pend = []   # deferred rs-broadcast + rope of the previous group

            def ph1(qraw_g, g0, gi):
                rrows = []
                for ct in range(4):
                    is_k = ct // 2
                    src_q = qraw_g[:, ct, :]
                    sq = p_sq.tile([128, 512], F16, tag="sq", name=f"sq{b}{gi}{ct}")
                    nc.vector.tensor_mul(out=sq[:], in0=src_q, in1=src_q)
                    ps_ss = p_ps.tile([1, 512], F32, tag="ps", name=f"ss{b}{gi}{ct}")
                    nc.tensor.matmul(ps_ss[:], invg2[:, is_k:is_k + 1], sq[:],
                                     start=True, stop=True)
                    # rrow = 1/sqrt(var+eps) = exp(-0.5*ln(var+eps))
                    lrow = p_row.tile([1, 512], F32, tag="lrow", name=f"lr{b}{gi}{ct}")
                    nc.scalar.activation(lrow[:], ps_ss[:],
                                         mybir.ActivationFunctionType.Ln,
                                         bias=eps_t[:], scale=1.0 / D)
                    rrow = p_row.tile([1, 512], F16, tag="recip", name=f"rr{b}{gi}{ct}")
                    nc.scalar.activation(rrow[:], lrow[:],
                                         mybir.ActivationFunctionType.Exp,
                                         scale=-0.5)
                    rrows.append(rrow)
                return rrows

            def ph2(qraw_g, g0, gi, rrows):
                for ct in range(4):
                    hl, is_k = ct % 2, ct // 2
                    dst = (kT if is_k else qT)
                    src_q = qraw_g[:, ct, :]
                    ps_rs = p_ps.tile([128, 512], F32, tag="ps", name=f"rs{b}{gi}{ct}")
                    nc.tensor.matmul(ps_rs[:], oner[:], rrows[ct][:],
                                     start=True, stop=True)
                    qsw = p_qsw.tile([128, 512], F16, tag="qsw", name=f"qsw{b}{gi}{ct}")
                    nc.sync.dma_start(qsw[0:64, :], src_q[64:128, :])
                    nc.sync.dma_start(qsw[64:128, :], src_q[0:64, :])
                    # in-place: qc into qraw, qs into qsw
                    nc.vector.tensor_mul(out=src_q, in0=src_q,
                                         in1=cos2[:, g0:g0 + 512])
                    nc.vector.tensor_mul(out=qsw[:], in0=qsw[:],
                                         in1=sin2[:, g0:g0 + 512])
                    rot = dst[:, hl, g0:g0 + 512]
                    nc.vector.tensor_add(out=rot, in0=src_q, in1=qsw[:])
                    nc.vector.tensor_mul(out=rot, in0=rot, in1=ps_rs[:])

            for ch in range(8):           # 256-token chunks
                tok0 = b * N + ch * 256
                if ch % 2 == 0:
                    qraw = p_qraw.tile([128, 4, 512], F16)
                off = (ch % 2) * 256
                ps_qk = [p_ps.tile([128, 256], F32, tag="ps", name=f"a{b}{ch}{ct}")
                         for ct in range(4)]
                ps_v = [p_ps.tile([128, 256], F32, tag="ps", name=f"av{b}{ch}{s}")
                        for s in range(2)]
                for half in range(2):
                    xt = p_xt.tile([128, 8, 256], F16)
                    src = xt_h.ap()[half * 1024:(half + 1) * 1024,
                                    tok0:tok0 + 256]
                    nc.sync.dma_start(xt[:], src.rearrange("(t p) j -> p t j", p=128))
                    for ct in range(4):
                        for kt in range(8):
                            nc.tensor.matmul(
                                ps_qk[ct][:], wqk[:, half * 8 + kt, ct * 128:(ct + 1) * 128],
                                xt[:, kt, :],
                                start=(half == 0 and kt == 0), stop=(half == 1 and kt == 7))
                    for s in range(2):
                        for kt in range(8):
                            nc.tensor.matmul(
                                ps_v[s][:], xt[:, kt, s * 128:(s + 1) * 128],
                                wv[:, half * 8 + kt, :],
                                start=(half == 0 and kt == 0), stop=(half == 1 and kt == 7))
                    # inject deferred epilogues mid-stream so their PE/ACT
                    # latency hides behind this chunk's dense matmuls
                    if half == 0 and ph1_pend:
                        args = ph1_pend.pop()
                        ph2_pend.append((args[0], args[1], args[2], ph1(*args)))
                    elif half == 1 and ph2_pend:
                        ph2(*ph2_pend.pop())
                for ct in range(4):
                    nc.vector.tensor_scalar_add(qraw[:, ct, off:off + 256],
                                                ps_qk[ct][:], qkb[:, ct:ct + 1])
                for s in range(2):
                    nc.vector.tensor_add(out=vtok[:, ch * 2 + s, :],
                                         in0=ps_v[s][:], in1=vb[:])
                if ch % 2 == 1:
                    ph1_pend.append((qraw, (ch - 1) * 256, ch // 2))
            # flush the last group's epilogue
            args = ph1_pend.pop()
            ph2(args[0], args[1], args[2], ph1(*args))

            # ======== SDPA for (b, h0) and (b, h1) ========
            ao = p_ao.tile([128, 2, N], F16)   # attn out^T, stays in SBUF
            pend = []                          # deferred normalize tails

            def normalize(hl, qc, ps_d, ps_av):
                q0 = qc * 512
                # ps_d holds the denominator replicated across all partitions
                # (ones-matrix lhsT); rd = 1/d = exp(-ln(d)) on 128 lanes.
                ld = p_attn.tile([128, 512], F32, tag="ld", name=f"ld{b}{hl}{qc}")
                nc.scalar.activation(ld[:], ps_d[:],
                                     mybir.ActivationFunctionType.Ln)
                rd = p_attn.tile([128, 512], F32, tag="rd", name=f"rd{b}{hl}{qc}")
                nc.scalar.activation(rd[:], ld[:],
                                     mybir.ActivationFunctionType.Exp,
                                     scale=-1.0)
                nc.vector.tensor_mul(out=ao[:, hl, q0:q0 + 512],
                                     in0=ps_av[:], in1=rd[:])

            for hl in range(HPC):
                for qc in range(4):       # 512-wide q chunks
                    q0 = qc * 512
                    ex = p_exp.tile([128, 16, 512], F16)
                    ps_d = p_ps.tile([128, 512], F32, tag="ps", name=f"d{b}{hl}{qc}")
                    ps_av = p_ps.tile([128, 512], F32, tag="ps", name=f"o{b}{hl}{qc}")
                    # software-pipelined: QK pairs run one pair ahead so ACT
                    # exp latency hides behind PE work (PE queue is in-order)
                    ps_s = [None] * 16

                    def qk(kt):
                        ps_s[kt] = p_ps.tile([128, 512], F32, tag="ps",
                                             name=f"s{b}{hl}{qc}{kt}")
                        nc.tensor.matmul(ps_s[kt][:], kT[:, hl, kt * 128:(kt + 1) * 128],
                                         qT[:, hl, q0:q0 + 512], start=True, stop=True)

                    def tailpair(j):
                        nc.scalar.activation(ex[:, j, :], ps_s[j][:],
                                             mybir.ActivationFunctionType.Exp,
                                             bias=nb4[:], scale=SCALE)
                        nc.scalar.activation(ex[:, j + 1, :], ps_s[j + 1][:],
                                             mybir.ActivationFunctionType.Exp,
                                             bias=nb4[:], scale=SCALE)
                        for kt in (j, j + 1):
                            nc.tensor.matmul(ps_av[:],
                                             vtok[:, kt, hl * 128:(hl + 1) * 128],
                                             ex[:, kt, :],
                                             start=(kt == 0), stop=(kt == 15))
                        for kt in (j, j + 1):
                            nc.tensor.matmul(ps_d[:], onec[:], ex[:, kt, :],
                                             start=(kt == 0), stop=(kt == 15))

                    for i in range(8):
                        qk(2 * i)
                        qk(2 * i + 1)
                        if i == 3 and pend:
                            normalize(*pend.pop())
                        if i >= 1:
                            tailpair(2 * i - 2)
                    tailpair(14)
                    pend.append((hl, qc, ps_d, ps_av))
            normalize(*pend.pop())

            # ======== stage C: partial proj for batch b ========
            for tt in range(16):          # 128-token tiles
                for oc in range(4):
                    ps_y = p_ps.tile([128, 512], F32, tag="ps", name=f"y{b}{tt}{oc}")
                    for ct in range(2):
                        nc.tensor.matmul(ps_y[:], ao[:, ct, tt * 128:(tt + 1) * 128],
                                         wp[:, ct, oc * 512:(oc + 1) * 512],
                                         start=(ct == 0), stop=(ct == 1))
                    yt = p_y.tile([128, 512], F32)
                    if oc % 2 == 0:
                        nc.vector.tensor_copy(yt[:], ps_y[:])
                    else:
                        nc.scalar.copy(yt[:], ps_y[:])
                    nc.sync.dma_start(
                        y_h.ap()[b * N + tt * 128:b * N + (tt + 1) * 128,
                                 oc * 512:(oc + 1) * 512], yt[:])

    nc.compile()
    _CACHE["nc"] = nc
    return nc


def make_in_maps(x, rope, qkv_w, qkv_b, proj_w, q_norm_w, k_norm_w):
    """Host-side prep: transpose x, slice/scale weights per core."""
    x = np.asarray(x, np.float32)
    rope = np.asarray(rope, np.float32)
    qkv_w = np.asarray(qkv_w, np.float32)
    qkv_b = np.asarray(qkv_b, np.float32)
    proj_w = np.asarray(proj_w, np.float32)
    g_q = np.asarray(q_norm_w, np.float32)
    g_k = np.asarray(k_norm_w, np.float32)
    if np.any(g_q == 0) or np.any(g_k == 0):
        raise ValueError("zero rmsnorm weight not supported")

    xt = np.ascontiguousarray(x.reshape(TOK, C).T.astype(np.float16))  # [C, TOK]
    cos = np.cos(rope)                                        # [N, 64]
    sin = np.sin(rope)
    cos2 = np.ascontiguousarray(
        np.concatenate([cos, cos], axis=1).T.astype(np.float16))       # [128, N]
    sin2 = np.ascontiguousarray(
        np.concatenate([-sin, sin], axis=1).T.astype(np.float16))      # [128, N]
    invg2 = np.stack([1.0 / g_q ** 2, 1.0 / g_k ** 2], axis=1).astype(np.float16)
    onec = np.ones((128, 128), np.float16)
    oner = np.ones((1, 128), np.float16)
    twor = np.full((1, 128), 2.0, np.float16)
    eps = np.full((1, 1), EPS, np.float32)
    nb4 = np.full((128, 1), ESHIFT, np.float32)

    in_maps = []
    for c in range(NCORES):
        hs = [HPC * c + hl for hl in range(HPC)]
        # chan-tiles: q_h0, q_h1, k_h0, k_h1 (g-scaled rows + bias)
        rows, biases = [], []
        for base, g in ((0, g_q), (C, g_k)):
            for h in hs:
                r0 = base + h * D
                rows.append(qkv_w[r0:r0 + D] * g[:, None])
                biases.append(qkv_b[r0:r0 + D] * g)
        wqk = np.ascontiguousarray(
            np.concatenate(rows, axis=0).T.astype(np.float16))           # [C, 512]
        qkb = np.stack(biases, axis=1)                                   # [128, 4]
        vrows = [qkv_w[2 * C + h * D:2 * C + (h + 1) * D] for h in hs]
        wv = np.ascontiguousarray(
            np.concatenate(vrows, axis=0).T.astype(np.float16))          # [C, 256]
        vbias = np.concatenate(
            [qkv_b[2 * C + h * D:2 * C + (h + 1) * D] for h in hs])      # [256]
        vb = np.broadcast_to(vbias, (128, 256)).astype(np.float32).copy()
        cols = np.concatenate([np.arange(h * D, (h + 1) * D) for h in hs])
        wpT = np.ascontiguousarray(proj_w[:, cols].T.astype(np.float16))  # [256, C]
        in_maps.append({
            "xt": xt, "wqk": wqk, "wv": wv, "wp": wpT,
            "cos2": cos2, "sin2": sin2, "qkb": qkb, "vb": vb,
            "invg2": invg2, "onec": onec, "oner": oner, "twor": twor,
            "eps": eps, "nb4": nb4,
        })
    return in_maps


def kernel(x, rope, qkv_w, qkv_b, proj_w, proj_b, q_norm_w, k_norm_w):
    nc = build_module()
    in_maps = make_in_maps(x, rope, qkv_w, qkv_b, proj_w, q_norm_w, k_norm_w)
    res = bass_utils.run_bass_kernel_spmd(nc, in_maps,
                                          core_ids=list(range(NCORES)), **RUN_KW)
    _CACHE["last_result"] = res
    y = np.zeros((TOK, C), np.float64)
    for c in range(NCORES):
        y += res.results[c]["y"].astype(np.float64)
    y += np.asarray(proj_b, np.float32).astype(np.float64)
    return y.astype(np.float32).reshape(B, N, C)


# revision 18
# speedup vs baseline: 1.4612x; 1.0437x over previous
"""Multi-head attention (16 heads, D=128) on 8 trn2 NeuronCores.

Sharding: tensor-parallel over heads — each core owns 2 heads.
Per core: qkv projection for its 768 channels (chan-major for q/k,
token-major for v), fused RMSNorm+RoPE on q/k, SDPA in transposed-score
layout (softmax partition reduction via ones-matmul on the PE), partial
proj over its 256 channels.  Host sums the 8 partial outputs + bias.

Matmul operands are fp16 (separate FWL weight load, full PE rate);
all accumulation is fp32 in PSUM; softmax statistics in fp32.
exp is computed as exp(s/sqrt(D) - 4) — the shift is softmax-invariant
and keeps fp16 exp values in range (no overflow).

Layouts (per core):
  xT       [C=2048, TOK=4096]  (x transposed on host; tokens = b*2048+n)
  w_qk     SBUF [128, 16, 512]  lhsT tiles; chan-tiles = [q_h0,q_h1,k_h0,k_h1]
  w_v      SBUF [128, 16, 256]  rhs tiles (token-major v production)
  qT/kT    SBUF [128, 2, 2048]  D-major per head, per batch
  v        SBUF [128, 16, 256]  token-major per batch
  exp      SBUF [128, 16, 512]  exp(scores^T) per 512-wide q-chunk
  attn_scr DRAM [256, 2048]     normalized attn out^T
  wpT      SBUF [128, 2, 2048]  proj rhs tiles
  y        DRAM [4096, 2048]    fp32 partial output (host adds cores + bias)
"""
import math
from contextlib import ExitStack

import numpy as np

import concourse.bass as bass
import concourse.mybir as mybir
import concourse.tile as tile
from concourse import bacc, bass_utils

F32 = mybir.dt.float32
F16 = mybir.dt.float16

H, D, B, N, C = 16, 128, 2, 2048, 2048
NCORES = 8
HPC = H // NCORES            # heads per core = 2
TOK = B * N                  # 4096
EPS = float(np.finfo(np.float32).eps)
SCALE = 1.0 / math.sqrt(D)
ESHIFT = -4.0                # exp(s*SCALE + ESHIFT); softmax-invariant

_CACHE = {}
RUN_KW = {}   # test.py sets {"trace": True}


def _pin_act_table():
    """Restrict Exp/Ln to the combined natural_log_exp_and_others set so the
    table-load pass keeps a single ACT table resident (the default greedy
    choice alternates exp_and_others <-> natural_log, ~2.7us per switch)."""
    import concourse.hw_specs as hw
    tabs = hw.get_activation_tables("gen3")
    for name, funcs in tabs.items():
        if name != "natural_log_exp_and_others":
            funcs.discard(mybir.ActivationFunctionType.Exp)
            funcs.discard(mybir.ActivationFunctionType.Ln)


def build_module():
    """Build + compile the per-core Bass module (same NEFF for all cores)."""
    if "nc" in _CACHE:
        return _CACHE["nc"]
    _pin_act_table()
    nc = bacc.Bacc("TRN2", target_bir_lowering=False, debug=False,
                   num_devices=NCORES)

    xt_h = nc.dram_tensor("xt", [C, TOK], F16, kind="ExternalInput")
    wqk_h = nc.dram_tensor("wqk", [C, 4 * 128], F16, kind="ExternalInput")
    wv_h = nc.dram_tensor("wv", [C, 2 * 128], F16, kind="ExternalInput")
    wp_h = nc.dram_tensor("wp", [2 * 128, C], F16, kind="ExternalInput")
    cos2_h = nc.dram_tensor("cos2", [128, N], F16, kind="ExternalInput")
    sin2_h = nc.dram_tensor("sin2", [128, N], F16, kind="ExternalInput")
    qkb_h = nc.dram_tensor("qkb", [128, 4], F32, kind="ExternalInput")
    vb_h = nc.dram_tensor("vb", [128, 256], F32, kind="ExternalInput")
    invg2_h = nc.dram_tensor("invg2", [128, 2], F16, kind="ExternalInput")
    onec_h = nc.dram_tensor("onec", [128, 128], F16, kind="ExternalInput")
    oner_h = nc.dram_tensor("oner", [1, 128], F16, kind="ExternalInput")
    twor_h = nc.dram_tensor("twor", [1, 128], F16, kind="ExternalInput")
    eps_h = nc.dram_tensor("eps", [1, 1], F32, kind="ExternalInput")
    nb4_h = nc.dram_tensor("nb4", [128, 1], F32, kind="ExternalInput")
    y_h = nc.dram_tensor("y", [TOK, C], F32, kind="ExternalOutput")

    with tile.TileContext(nc) as tc, ExitStack() as ctx:
        pc = ctx.enter_context(tc.tile_pool(name="consts", bufs=1))
        p_xt = ctx.enter_context(tc.tile_pool(name="xt", bufs=3))
        p_qkv = ctx.enter_context(tc.tile_pool(name="qkv", bufs=1))
        p_qraw = ctx.enter_context(tc.tile_pool(name="qraw", bufs=2))
        p_qsw = ctx.enter_context(tc.tile_pool(name="qsw", bufs=3))
        p_sq = ctx.enter_context(tc.tile_pool(name="sq", bufs=3))
        p_exp = ctx.enter_context(tc.tile_pool(name="exp", bufs=2))
        p_attn = ctx.enter_context(tc.tile_pool(name="attn", bufs=4))
        p_ao = ctx.enter_context(tc.tile_pool(name="ao", bufs=1))
        p_y = ctx.enter_context(tc.tile_pool(name="y", bufs=8))
        p_row = ctx.enter_context(tc.tile_pool(name="rows", bufs=4))
        p_ps = ctx.enter_context(tc.tile_pool(name="ps", bufs=8, space="PSUM"))

        # constants / weights (resident)
        wqk = pc.tile([128, 16, 512], F16)
        wv = pc.tile([128, 16, 256], F16)
        for hf in range(2):
            sl = slice(hf * 1024, (hf + 1) * 1024)
            nc.sync.dma_start(wqk[:, hf * 8:(hf + 1) * 8, :],
                              wqk_h.ap()[sl].rearrange("(t p) j -> p t j", p=128))
            nc.sync.dma_start(wv[:, hf * 8:(hf + 1) * 8, :],
                              wv_h.ap()[sl].rearrange("(t p) j -> p t j", p=128))
        wp = pc.tile([128, 2, 2048], F16)
        nc.sync.dma_start(wp[:], wp_h.ap().rearrange("(t p) j -> p t j", p=128))
        cos2 = pc.tile([128, N], F16)
        nc.sync.dma_start(cos2[:], cos2_h.ap())
        sin2 = pc.tile([128, N], F16)
        nc.sync.dma_start(sin2[:], sin2_h.ap())
        qkb = pc.tile([128, 4], F32)
        nc.sync.dma_start(qkb[:], qkb_h.ap())
        vb = pc.tile([128, 256], F32)
        nc.sync.dma_start(vb[:], vb_h.ap())
        invg2 = pc.tile([128, 2], F16)
        nc.sync.dma_start(invg2[:], invg2_h.ap())
        onec = pc.tile([128, 128], F16)
        nc.sync.dma_start(onec[:], onec_h.ap())
        oner = pc.tile([1, 128], F16)
        nc.sync.dma_start(oner[:], oner_h.ap())
        twor = pc.tile([1, 128], F16)
        nc.sync.dma_start(twor[:], twor_h.ap())
        eps_t = pc.tile([1, 1], F32)
        nc.sync.dma_start(eps_t[:], eps_h.ap())
        nb4 = pc.tile([128, 1], F32)
        nc.sync.dma_start(nb4[:], nb4_h.ap())

        for b in range(B):
            # ======== stage A: qkv projection for batch b ========
            qT = p_qkv.tile([128, HPC, N], F16, tag="qT")
            kT = p_qkv.tile([128, HPC, N], F16, tag="kT")
            vtok = p_qkv.tile([128, 16, 256], F16, tag="v")
            qraw = None
            ph1_pend = []   # deferred sq/sumsq/ln/exp of the previous group
            ph2_pend = []   # deferred rs-broadcast + rope of the previous group

            def ph1(qraw_g, g0, gi):
                rrows = []
                for ct in range(4):
                    is_k = ct // 2
                    src_q = qraw_g[:, ct, :]
                    sq = p_sq.tile([128, 512], F16, tag="sq", name=f"sq{b}{gi}{ct}")
                    nc.vector.tensor_mul(out=sq[:], in0=src_q, in1=src_q)
                    ps_ss = p_ps.tile([1, 512], F32, tag="ps", name=f"ss{b}{gi}{ct}")
                    nc.tensor.matmul(ps_ss[:], invg2[:, is_k:is_k + 1], sq[:],
                                     start=True, stop=True)
                    # rrow = 1/sqrt(var+eps) = exp(-0.5*ln(var+eps))
                    lrow = p_row.tile([1, 512], F32, tag="lrow", name=f"lr{b}{gi}{ct}")
                    nc.scalar.activation(lrow[:], ps_ss[:],
                                         mybir.ActivationFunctionType.Ln,
                                         bias=eps_t[:], scale=1.0 / D)
                    rrow = p_row.tile([1, 512], F16, tag="recip", name=f"rr{b}{gi}{ct}")
                    nc.scalar.activation(rrow[:], lrow[:],
                                         mybir.ActivationFunctionType.Exp,
                                         scale=-0.5)
                    rrows.append(rrow)
                return rrows

            def ph2(qraw_g, g0, gi, rrows):
                for ct in range(4):
                    hl, is_k = ct % 2, ct // 2
                    dst = (kT if is_k else qT)
                    src_q = qraw_g[:, ct, :]
                    ps_rs = p_ps.tile([128, 512], F32, tag="ps", name=f"rs{b}{gi}{ct}")
                    nc.tensor.matmul(ps_rs[:], oner[:], rrows[ct][:],
                                     start=True, stop=True)
                    qsw = p_qsw.tile([128, 512], F16, tag="qsw", name=f"qsw{b}{gi}{ct}")
                    nc.sync.dma_start(qsw[0:64, :], src_q[64:128, :])
                    nc.sync.dma_start(qsw[64:128, :], src_q[0:64, :])
                    # in-place: qc into qraw, qs into qsw
                    nc.vector.tensor_mul(out=src_q, in0=src_q,
                                         in1=cos2[:, g0:g0 + 512])
                    nc.vector.tensor_mul(out=qsw[:], in0=qsw[:],
                                         in1=sin2[:, g0:g0 + 512])
                    rot = dst[:, hl, g0:g0 + 512]
                    nc.vector.tensor_add(out=rot, in0=src_q, in1=qsw[:])
                    nc.vector.tensor_mul(out=rot, in0=rot, in1=ps_rs[:])

            for ch in range(8):           # 256-token chunks
                tok0 = b * N + ch * 256
                if ch % 2 == 0:
                    qraw = p_qraw.tile([128, 4, 512], F16)
                off = (ch % 2) * 256
                ps_qk = [p_ps.tile([128, 256], F32, tag="ps", name=f"a{b}{ch}{ct}")
                         for ct in range(4)]
                ps_v = [p_ps.tile([128, 256], F32, tag="ps", name=f"av{b}{ch}{s}")
                        for s in range(2)]
                for half in range(2):
                    xt = p_xt.tile([128, 8, 256], F16)
                    src = xt_h.ap()[half * 1024:(half + 1) * 1024,
                                    tok0:tok0 + 256]
                    nc.sync.dma_start(xt[:], src.rearrange("(t p) j -> p t j", p=128))
                    for ct in range(4):
                        for kt in range(8):
                            nc.tensor.matmul(
                                ps_qk[ct][:], wqk[:, half * 8 + kt, ct * 128:(ct + 1) * 128],
                                xt[:, kt, :],
                                start=(half == 0 and kt == 0), stop=(half == 1 and kt == 7))
                    for s in range(2):
                        for kt in range(8):
                            nc.tensor.matmul(
                                ps_v[s][:], xt[:, kt, s * 128:(s + 1) * 128],
                                wv[:, half * 8 + kt, :],
                                start=(half == 0 and kt == 0), stop=(half == 1 and kt == 7))
                    # inject deferred epilogues mid-stream so their PE/ACT
                    # latency hides behind this chunk's dense matmuls
                    if half == 0 and ph1_pend:
                        args = ph1_pend.pop()
                        ph2_pend.append((args[0], args[1], args[2], ph1(*args)))
                    elif half == 1 and ph2_pend:
                        ph2(*ph2_pend.pop())
                for ct in range(4):
                    nc.vector.tensor_scalar_add(qraw[:, ct, off:off + 256],
                                                ps_qk[ct][:], qkb[:, ct:ct + 1])
                for s in range(2):
                    nc.vector.tensor_add(out=vtok[:, ch * 2 + s, :],
                                         in0=ps_v[s][:], in1=vb[:])
                if ch % 2 == 1:
                    ph1_pend.append((qraw, (ch - 1) * 256, ch // 2))
            # flush the last group's epilogue
            args = ph1_pend.pop()
            ph2(args[0], args[1], args[2], ph1(*args))

            # ======== SDPA for (b, h0) and (b, h1) ========
            ao = p_ao.tile([128, 2, N], F16)   # attn out^T, stays in SBUF
            pend = []                          # deferred normalize tails

            def normalize(hl, qc, ps_d, ps_av):
                q0 = qc * 512
                # ps_d holds the denominator replicated across all partitions
                # (ones-matrix lhsT); rd = 1/d = exp(-ln(d)) on 128 lanes.
                ld = p_attn.tile([128, 512], F32, tag="ld", name=f"ld{b}{hl}{qc}")
                nc.scalar.activation(ld[:], ps_d[:],
                                     mybir.ActivationFunctionType.Ln)
                rd = p_attn.tile([128, 512], F32, tag="rd", name=f"rd{b}{hl}{qc}")
                nc.scalar.activation(rd[:], ld[:],
                                     mybir.ActivationFunctionType.Exp,
                                     scale=-1.0)
                nc.vector.tensor_mul(out=ao[:, hl, q0:q0 + 512],
                                     in0=ps_av[:], in1=rd[:])

            for hl in range(HPC):
                for qc in range(4):       # 512-wide q chunks
                    q0 = qc * 512
                    ex = p_exp.tile([128, 16, 512], F16)
                    # software-pipelined: QK pairs run one pair ahead so ACT
                    # exp latency hides behind PE work (PE queue is in-order)
                    ps_s = [None] * 16
                    ps_d = ps_av = None

                    def qk(kt):
                        ps_s[kt] = p_ps.tile([128, 512], F32, tag="ps",
                                             name=f"s{b}{hl}{qc}{kt}")
                        nc.tensor.matmul(ps_s[kt][:], kT[:, hl, kt * 128:(kt + 1) * 128],
                                         qT[:, hl, q0:q0 + 512], start=True, stop=True)

                    def tailpair(j):
                        nc.scalar.activation(ex[:, j, :], ps_s[j][:],
                                             mybir.ActivationFunctionType.Exp,
                                             bias=nb4[:], scale=SCALE)
                        nc.scalar.activation(ex[:, j + 1, :], ps_s[j + 1][:],
                                             mybir.ActivationFunctionType.Exp,
                                             bias=nb4[:], scale=SCALE)
                        for kt in (j, j + 1):
                            nc.tensor.matmul(ps_av[:],
                                             vtok[:, kt, hl * 128:(hl + 1) * 128],
                                             ex[:, kt, :],
                                             start=(kt == 0), stop=(kt == 15))
                        for kt in (j, j + 1):
                            nc.tensor.matmul(ps_d[:], onec[:], ex[:, kt, :],
                                             start=(kt == 0), stop=(kt == 15))

                    for i in range(8):
                        qk(2 * i)
                        qk(2 * i + 1)
                        if i == 1:
                            if pend:
                                normalize(*pend.pop())
                            ps_d = p_ps.tile([128, 512], F32, tag="ps",
                                             name=f"d{b}{hl}{qc}")
                            ps_av = p_ps.tile([128, 512], F32, tag="ps",
                                              name=f"o{b}{hl}{qc}")
                        if i >= 1:
                            tailpair(2 * i - 2)
                    tailpair(14)
                    pend.append((hl, qc, ps_d, ps_av))
            normalize(*pend.pop())

            # ======== stage C: partial proj for batch b ========
            for tt in range(16):          # 128-token tiles
                for oc in range(4):
                    ps_y = p_ps.tile([128, 512], F32, tag="ps", name=f"y{b}{tt}{oc}")
                    for ct in range(2):
                        nc.tensor.matmul(ps_y[:], ao[:, ct, tt * 128:(tt + 1) * 128],
                                         wp[:, ct, oc * 512:(oc + 1) * 512],
                                         start=(ct == 0), stop=(ct == 1))
                    yt = p_y.tile([128, 512], F32)
                    if oc % 2 == 0:
                        nc.vector.tensor_copy(yt[:], ps_y[:])
                    else:
                        nc.scalar.copy(yt[:], ps_y[:])
                    nc.sync.dma_start(
                        y_h.ap()[b * N + tt * 128:b * N + (tt + 1) * 128,
                                 oc * 512:(oc + 1) * 512], yt[:])

    nc.compile()
    _CACHE["nc"] = nc
    return nc


def make_in_maps(x, rope, qkv_w, qkv_b, proj_w, q_norm_w, k_norm_w):
    """Host-side prep: transpose x, slice/scale weights per core."""
    x = np.asarray(x, np.float32)
    rope = np.asarray(rope, np.float32)
    qkv_w = np.asarray(qkv_w, np.float32)
    qkv_b = np.asarray(qkv_b, np.float32)
    proj_w = np.asarray(proj_w, np.float32)
    g_q = np.asarray(q_norm_w, np.float32)
    g_k = np.asarray(k_norm_w, np.float32)
    if np.any(g_q == 0) or np.any(g_k == 0):
        raise ValueError("zero rmsnorm weight not supported")

    xt = np.ascontiguousarray(x.reshape(TOK, C).T.astype(np.float16))  # [C, TOK]
    cos = np.cos(rope)                                        # [N, 64]
    sin = np.sin(rope)
    cos2 = np.ascontiguousarray(
        np.concatenate([cos, cos], axis=1).T.astype(np.float16))       # [128, N]
    sin2 = np.ascontiguousarray(
        np.concatenate([-sin, sin], axis=1).T.astype(np.float16))      # [128, N]
    invg2 = np.stack([1.0 / g_q ** 2, 1.0 / g_k ** 2], axis=1).astype(np.float16)
    onec = np.ones((128, 128), np.float16)
    oner = np.ones((1, 128), np.float16)
    twor = np.full((1, 128), 2.0, np.float16)
    eps = np.full((1, 1), EPS, np.float32)
    nb4 = np.full((128, 1), ESHIFT, np.float32)

    in_maps = []
    for c in range(NCORES):
        hs = [HPC * c + hl for hl in range(HPC)]
        # chan-tiles: q_h0, q_h1, k_h0, k_h1 (g-scaled rows + bias)
        rows, biases = [], []
        for base, g in ((0, g_q), (C, g_k)):
            for h in hs:
                r0 = base + h * D
                rows.append(qkv_w[r0:r0 + D] * g[:, None])
                biases.append(qkv_b[r0:r0 + D] * g)
        wqk = np.ascontiguousarray(
            np.concatenate(rows, axis=0).T.astype(np.float16))           # [C, 512]
        qkb = np.stack(biases, axis=1)                                   # [128, 4]
        vrows = [qkv_w[2 * C + h * D:2 * C + (h + 1) * D] for h in hs]
        wv = np.ascontiguousarray(
            np.concatenate(vrows, axis=0).T.astype(np.float16))          # [C, 256]
        vbias = np.concatenate(
            [qkv_b[2 * C + h * D:2 * C + (h + 1) * D] for h in hs])      # [256]
        vb = np.broadcast_to(vbias, (128, 256)).astype(np.float32).copy()
        cols = np.concatenate([np.arange(h * D, (h + 1) * D) for h in hs])
        wpT = np.ascontiguousarray(proj_w[:, cols].T.astype(np.float16))  # [256, C]
        in_maps.append({
            "xt": xt, "wqk": wqk, "wv": wv, "wp": wpT,
            "cos2": cos2, "sin2": sin2, "qkb": qkb, "vb": vb,
            "invg2": invg2, "onec": onec, "oner": oner, "twor": twor,
            "eps": eps, "nb4": nb4,
        })
    return in_maps


def kernel(x, rope, qkv_w, qkv_b, proj_w, proj_b, q_norm_w, k_norm_w):
    nc = build_module()
    in_maps = make_in_maps(x, rope, qkv_w, qkv_b, proj_w, q_norm_w, k_norm_w)
    res = bass_utils.run_bass_kernel_spmd(nc, in_maps,
                                          core_ids=list(range(NCORES)), **RUN_KW)
    _CACHE["last_result"] = res
    y = np.zeros((TOK, C), np.float64)
    for c in range(NCORES):
        y += res.results[c]["y"].astype(np.float64)
    y += np.asarray(proj_b, np.float32).astype(np.float64)
    return y.astype(np.float32).reshape(B, N, C)
